# revision 20
# baseline (speedup 1.0000x reference)
"""Trainium2 Bass kernel v3: LiquidCell (Euler scan over 3-layer MLP+LN).

Changes vs v2 (4.10 ms):
- tm(t=0)=tanh(0)=0: the first Euler step contributes nothing -> run only
  the 8 steps with tm != 0 (host detects zero steps generically).
- Stale-R LayerNorm: normalize each step's L1 PSUM output directly with the
  previous step's 1/sigma (one fused DVE multiply per PSUM group replaces the
  drain + separate normalize passes), then refresh R afterward with the same
  self-correcting Newton form R' = R*(1.5 - 0.5*E[(z*R)^2]).  numpy sim of
  the full pipeline: rel_l2 1.2e-3 (v2 measured 2.2e-3).
- tanh(t*wt)*dt folded into per-step fp8 W3 copies -> h-update is one merged
  scalar_tensor_tensor with an immediate scale; no tmdtp tensor.
- Bigger instructions: one N=4096 gelu for L1, two N=1024 gelus for L2,
  N=1024 fused drains / h-update (PSUM tiles span 2 banks) to amortize the
  ~352-cycle ACT and ~130-cycle DVE fixed costs.
- Variance matmul: sampled squares stored fp8 (scale 1/16) so the reduction
  is a single DoubleRow fp8 matmul.
- Engine balance per tile-step: PE ~8.0us, DVE ~7.2us, ACT ~6.7us,
  GPSIMD ~5.6us.
"""

import numpy as np

P = 128
NCORES = 8
BT = 512          # batch tile (matmul free dim)
G = 4             # tiles interleaved per loop body
LN_EPS = 1e-5
A1 = 16.0         # host scale folded into W1 (cancels in LN)
A2 = 16.0         # host scale folded into W2 (divided out in gelu2 scale)
SS_CH = 2         # z1 chunks sampled for the LN variance estimate (of 8)
LDW_OPT = False   # --enable-ldw-opt=true crashes walrus codegen (CoreV3GenImpl)


def _emit(nc, tc, d, BC, NE, hsc, flags):
    import concourse.mybir as mybir
    from concourse.bass import ds, ts
    from contextlib import ExitStack

    f32 = mybir.dt.float32
    bf16 = mybir.dt.bfloat16
    fp8 = mybir.dt.float8e4
    AF = mybir.ActivationFunctionType
    OP = mybir.AluOpType
    ET = mybir.EngineType
    DR = mybir.MatmulPerfMode.DoubleRow

    has_b1 = flags["has_b1"]
    has_aff1 = flags["has_aff1"]
    has_b2 = flags["has_b2"]
    has_b3 = flags["has_b3"]
    has_affo = flags["has_affo"]

    n_m1 = 8                  # 4H/P
    n_gr = 4                  # L1 PSUM groups of 2 chunks
    n_m2 = 4                  # 2H/P
    n_c = 2                   # H/P

    with ExitStack() as ctx:
        singles = ctx.enter_context(tc.tile_pool(name="singles", bufs=1))
        pool_io = ctx.enter_context(tc.tile_pool(name="io", bufs=2 * G))
        pool_big = ctx.enter_context(tc.tile_pool(name="big", bufs=G))
        pool_w = ctx.enter_context(tc.tile_pool(name="wrk", bufs=4))
        pool_fin = ctx.enter_context(tc.tile_pool(name="fin", bufs=2))
        pool_row = ctx.enter_context(tc.tile_pool(name="rows", bufs=2))
        psA = ctx.enter_context(tc.tile_pool(name="psA", bufs=2, space="PSUM"))
        psB = ctx.enter_context(tc.tile_pool(name="psB", bufs=2, space="PSUM"))

        def load(name, dtype):
            t = singles.tile(list(d[name].shape), dtype, tag=name)
            nc.sync.dma_start(out=t, in_=d[name][:])
            return t

        w1t8 = load("w1t8", fp8)      # [128, 2, 2, 1024]
        w2t8 = load("w2t8", fp8)      # [128, 4, 2, 512]
        w3t8 = load("w3t8", fp8)      # [128, NE, 2, 2, 256]
        b1cp = load("b1cp", f32) if has_b1 else None
        g1p = load("g1p", f32) if has_aff1 else None
        be1p = load("be1p", f32) if has_aff1 else None
        b2p = load("b2p", f32) if has_b2 else None
        b3row = load("b3row", bf16) if has_b3 else None
        goutp = load("goutp", f32) if has_affo else None
        beoutp = load("beoutp", f32) if has_affo else None

        # seed-step variance row: vrow = A1^2 * sigma^2 over SS_CH*P features
        ones_ss = singles.tile([P, 1], bf16)
        nc.vector.memset(ones_ss, 1.0 / (SS_CH * P))
        # steady-state: q01 holds u^2/16; allones8 (fp8 DR lhsT) of 1/16 makes
        # bcv[p,j] = (1/256) sum_{256} u^2 = vhat (the 16s cancel)
        allones8 = singles.tile([P, 2, P], fp8)
        nc.vector.memset(allones8, 1.0 / 16.0)
        onerow = singles.tile([1, P], bf16)
        nc.vector.memset(onerow, 1.0)
        ones_hb = singles.tile([P, 1], bf16)
        nc.vector.memset(ones_hb, 1.0 / (n_c * P))
        eps_t = singles.tile([1, 1], f32)
        nc.vector.memset(eps_t, LN_EPS * A1 * A1)
        eps_o = singles.tile([1, 1], f32)
        nc.vector.memset(eps_o, LN_EPS)
        c15 = singles.tile([P, 1], f32)
        nc.vector.memset(c15, 1.5)
        if has_b3:
            onesrow_bf = singles.tile([1, BT], bf16)
            nc.vector.memset(onesrow_bf, 1.0)

        class T:
            pass

        def make_tile(off, j):
            t = T()
            t.off = off
            t.j = j
            t.x8 = pool_io.tile([P, 2, BT], fp8, tag="x8")
            t.hT = pool_io.tile([P, 2, BT], f32, tag="hT")
            t.h8 = pool_io.tile([P, 2, BT], fp8, tag="h8")
            t.u = pool_big.tile([P, n_m1, BT], bf16, tag="u")
            t.z1g = pool_big.tile([P, n_m1, BT], fp8, tag="z1g")
            t.z2 = pool_big.tile([P, n_m2, BT], fp8, tag="z2")
            t.q01 = pool_big.tile([P, 2, BT], fp8, tag="q01")
            t.rsB = pool_big.tile([P, BT], bf16, tag="rsB")
            nc.sync.dma_start(out=t.x8, in_=d["x8"][:, :, ds(off, BT)])
            nc.sync.dma_start(out=t.hT, in_=d["hf"][:, :, ds(off, BT)])
            nc.sync.dma_start(out=t.h8, in_=d["h8"][:, :, ds(off, BT)])
            return t

        def l1_mms(t, p):
            # one PSUM group = out-chunks 2p, 2p+1; h-pass + x-pass per chunk
            zps = psA.tile([P, 2, BT], f32, tag="l1")
            for c in range(2):
                m = 2 * p + c
                nc.tensor.matmul(zps[:, c, :], lhsT=w1t8[:, 0, :, ts(m, P)],
                                 rhs=t.h8, start=True, stop=False, perf_mode=DR)
                nc.tensor.matmul(zps[:, c, :], lhsT=w1t8[:, 1, :, ts(m, P)],
                                 rhs=t.x8, start=False, stop=True, perf_mode=DR)
            return zps

        def fused_drain(t, p, zps):
            # u = (z + b1) * R  with R from the previous step (stale)
            sl = slice(2 * p, 2 * p + 2)
            rb = t.rsB[:, None, :].to_broadcast((P, 2, BT))
            if has_b1:
                for c in range(2):
                    m = 2 * p + c
                    nc.vector.scalar_tensor_tensor(
                        out=t.u[:, m, :], in0=zps[:, c, :],
                        scalar=b1cp[:, m:m + 1], in1=t.rsB,
                        op0=OP.add, op1=OP.mult)
            else:
                nc.vector.tensor_mul(out=t.u[:, sl, :], in0=zps, in1=rb)

        def sq(t):
            # q01 = u^2/16 (fp8); sampled squares for the variance estimate
            nc.vector.scalar_tensor_tensor(
                out=t.q01, in0=t.u[:, 0:2, :], scalar=1.0 / 16.0,
                in1=t.u[:, 0:2, :], op0=OP.mult, op1=OP.mult)

        def pWB(t):
            # emitted one slot after sq: bcv never blocks the PE FIFO and
            # w is the first ACT op of the slot
            bcv = psB.tile([P, 2, BT], f32, tag="ps2")
            nc.tensor.matmul(bcv[:, 0, :], lhsT=allones8, rhs=t.q01,
                             start=True, stop=True, perf_mode=DR)
            # w = 1.5 - 0.5*vhat ; R <- R*w   (one-Newton rsqrt at seed 1)
            t.wP = pool_w.tile([P, BT], bf16, tag="wP")
            nc.scalar.activation(out=t.wP, in_=bcv[:, 0, :],
                                 func=AF.Identity, bias=c15[:, 0:1],
                                 scale=-0.5)

        def r_mul(t):
            nc.gpsimd.tensor_mul(out=t.rsB, in0=t.rsB, in1=t.wP)

        def gelu1(t):
            if has_aff1:
                for m in range(n_m1):
                    nc.scalar.activation(out=t.z1g[:, m, :], in_=t.u[:, m, :],
                                         func=AF.Gelu, bias=be1p[:, m:m + 1],
                                         scale=g1p[:, m:m + 1])
            else:
                nc.scalar.activation(out=t.z1g, in_=t.u, func=AF.Gelu,
                                     bias=0.0, scale=1.0)

        def pA_seed(t):
            # first real step: raw drains, exact ln/exp 1/sigma, normalize
            for p in range(n_gr):
                zps = l1_mms(t, p)
                if p % 2 == 0:
                    nc.scalar.copy(out=t.u[:, 2 * p:2 * p + 2, :], in_=zps)
                else:
                    nc.vector.tensor_copy(out=t.u[:, 2 * p:2 * p + 2, :],
                                          in_=zps)
            q01b = pool_fin.tile([P, 2, BT], bf16, tag="q0b")
            nc.vector.tensor_mul(out=q01b, in0=t.u[:, 0:2, :],
                                 in1=t.u[:, 0:2, :])
            vps = psB.tile([P, 2, BT], f32, tag="ps2")
            for m in range(SS_CH):
                nc.tensor.matmul(vps[0:1, 0, :], lhsT=ones_ss,
                                 rhs=q01b[:, m, :], start=(m == 0),
                                 stop=(m == SS_CH - 1))
            lnv = pool_row.tile([1, BT], f32, tag="lnv")
            nc.scalar.activation(out=lnv, in_=vps[0:1, 0, :], func=AF.Ln,
                                 bias=eps_t[0:1, 0:1], scale=1.0)
            rsr = pool_row.tile([1, BT], bf16, tag="rsr")
            nc.scalar.activation(out=rsr, in_=lnv, func=AF.Exp,
                                 bias=0.0, scale=-0.5)
            bc = psB.tile([P, 2, BT], f32, tag="ps2")
            nc.tensor.matmul(bc[:, 0, :], lhsT=onerow, rhs=rsr,
                             start=True, stop=True)
            nc.vector.tensor_copy(out=t.rsB, in_=bc[:, 0, :])
            rb = t.rsB[:, None, :].to_broadcast((P, 4, BT))
            for hf in range(2):
                hs = slice(4 * hf, 4 * hf + 4)
                if has_b1:
                    for m in range(4 * hf, 4 * hf + 4):
                        nc.vector.scalar_tensor_tensor(
                            out=t.u[:, m, :], in0=t.u[:, m, :],
                            scalar=b1cp[:, m:m + 1], in1=t.rsB,
                            op0=OP.add, op1=OP.mult)
                else:
                    nc.vector.tensor_mul(out=t.u[:, hs, :], in0=t.u[:, hs, :],
                                         in1=rb)
            gelu1(t)

        def pA(t, seed, last):
            if seed:
                pA_seed(t)
                return
            for p in range(n_gr):
                zps = l1_mms(t, p)
                fused_drain(t, p, zps)
                if p == 0 and not last:
                    sq(t)
            gelu1(t)

        def pBC(t, i, seed, last):
            # L2 in two halves, straight from PSUM through gelu2
            for h in range(2):
                zps = psB.tile([P, 2, BT], f32, tag="ps2")
                for c in range(2):
                    m = 2 * h + c
                    for p8 in range(4):
                        nc.tensor.matmul(zps[:, c, :],
                                         lhsT=w2t8[:, p8, :, ts(m, P)],
                                         rhs=t.z1g[:, 2 * p8:2 * p8 + 2, :],
                                         start=(p8 == 0), stop=(p8 == 3),
                                         perf_mode=DR)
                if has_b2:
                    for c in range(2):
                        m = 2 * h + c
                        nc.scalar.activation(out=t.z2[:, m, :],
                                             in_=zps[:, c, :], func=AF.Gelu,
                                             bias=b2p[:, m:m + 1],
                                             scale=1.0 / A2)
                else:
                    nc.scalar.activation(out=t.z2[:, 2 * h:2 * h + 2, :],
                                         in_=zps, func=AF.Gelu,
                                         bias=0.0, scale=1.0 / A2)
            # L3 with per-step W3*tm*dt folded weights
            dps = psB.tile([P, 2, BT], f32, tag="ps2")
            for c in range(n_c):
                for p4 in range(2):
                    nc.tensor.matmul(dps[:, c, :],
                                     lhsT=w3t8[:, i, p4, :, ts(c, P)],
                                     rhs=t.z2[:, 2 * p4:2 * p4 + 2, :],
                                     start=(p4 == 0),
                                     stop=(p4 == 1) and not has_b3,
                                     perf_mode=DR)
                if has_b3:
                    nc.tensor.matmul(dps[:, c, :], lhsT=b3row[0:1, ts(c, P)],
                                     rhs=onesrow_bf, start=False, stop=True)
            nc.vector.scalar_tensor_tensor(
                out=t.hT, in0=dps, scalar=hsc[i], in1=t.hT,
                op0=OP.mult, op1=OP.add)
            if not last:
                nc.gpsimd.tensor_copy(out=t.h8, in_=t.hT)
                if not seed:
                    r_mul(t)

        def pfinal(t):
            hb = pool_fin.tile([P, 2, BT], bf16, tag="hb")
            nc.gpsimd.tensor_copy(out=hb, in_=t.hT)
            q2 = pool_fin.tile([P, 2, BT], bf16, tag="q2")
            nc.vector.tensor_mul(out=q2, in0=hb, in1=hb)
            rows = psB.tile([P, 2, BT], f32, tag="ps2")
            for c in range(n_c):
                nc.tensor.matmul(rows[0:1, 0, :], lhsT=ones_hb,
                                 rhs=hb[:, c, :], start=(c == 0),
                                 stop=(c == n_c - 1))
            for c in range(n_c):
                nc.tensor.matmul(rows[0:1, 1, :], lhsT=ones_hb,
                                 rhs=q2[:, c, :], start=(c == 0),
                                 stop=(c == n_c - 1))
            mu2 = pool_row.tile([1, BT], f32, tag="mu2")
            nc.vector.tensor_copy(out=mu2, in_=rows[0:1, 0, :])
            var2 = pool_row.tile([1, BT], f32, tag="var2")
            msq = pool_row.tile([1, BT], f32, tag="msq")
            nc.vector.tensor_mul(out=msq, in0=mu2, in1=mu2)
            nc.vector.tensor_sub(out=var2, in0=rows[0:1, 1, :], in1=msq)
            lnv2 = pool_row.tile([1, BT], f32, tag="lnv2")
            nc.scalar.activation(out=lnv2, in_=var2, func=AF.Ln,
                                 bias=eps_o[0:1, 0:1], scale=1.0)
            rs2 = pool_row.tile([1, BT], bf16, tag="rs2")
            nc.scalar.activation(out=rs2, in_=lnv2, func=AF.Exp,
                                 bias=0.0, scale=-0.5)
            mu2b = pool_row.tile([1, BT], bf16, tag="mu2b")
            nc.vector.tensor_copy(out=mu2b, in_=mu2)
            bmrs = psB.tile([P, 2, BT], f32, tag="ps2")
            nc.tensor.matmul(bmrs[:, 0, :], lhsT=onerow, rhs=mu2b,
                             start=True, stop=True)
            nc.tensor.matmul(bmrs[:, 1, :], lhsT=onerow, rhs=rs2,
                             start=True, stop=True)
            dd = pool_fin.tile([P, 2, BT], f32, tag="dd")
            nc.vector.tensor_sub(
                out=dd, in0=t.hT,
                in1=bmrs[:, 0:1, :].to_broadcast((P, 2, BT)))
            nc.vector.tensor_mul(
                out=dd, in0=dd,
                in1=bmrs[:, 1:2, :].to_broadcast((P, 2, BT)))
            if has_affo:
                oc = pool_fin.tile([P, 2, BT], f32, tag="oc")
                for c in range(n_c):
                    nc.scalar.activation(out=oc[:, c, :], in_=dd[:, c, :],
                                         func=AF.Identity,
                                         bias=beoutp[:, c:c + 1],
                                         scale=goutp[:, c:c + 1])
                dd = oc
            nc.scalar.dma_start(out=d["out"][:, :, ds(t.off, BT)], in_=dd)

        REP = 2 if BC % (2 * G * BT) == 0 else 1
        assert BC % (REP * G * BT) == 0
        with tc.For_i(0, BC, REP * G * BT,
                      hint_engines=(ET.PE, ET.Activation, ET.DVE)) as i0:
            # flat pipeline over REP*NE*G tile-slots: pBC lags pA by 2 slots,
            # pWB by 1, so each step's L2/L3 tail interleaves with the next
            # step's L1 and the h-update STTs never sit ahead of the next
            # drains in DVE's FIFO.  REP=2 halves the loop-wrap bubble.
            slots = [(r, i, j) for r in range(REP)
                     for i in range(NE) for j in range(G)]
            tiles = {}
            for k in range(len(slots) + 2):
                if k < len(slots):
                    r, i, j = slots[k]
                    if i == 0:
                        tiles[(r, j)] = make_tile(
                            i0 if (r, j) == (0, 0)
                            else i0 + (r * G + j) * BT, j)
                    pA(tiles[(r, j)], i == 0, i == NE - 1)
                if 1 <= k <= len(slots):
                    r, i, j = slots[k - 1]
                    if 0 < i < NE - 1:
                        pWB(tiles[(r, j)])
                if k >= 2:
                    r, i, j = slots[k - 2]
                    pBC(tiles[(r, j)], i, i == 0, i == NE - 1)
                    if i == NE - 1:
                        pfinal(tiles[(r, j)])


def build_nc(BC, NE, hsc, flags, shapes):
    import concourse.bass as bass
    import concourse.mybir as mybir
    import concourse.tile as tile

    f32 = mybir.dt.float32
    bf16 = mybir.dt.bfloat16
    fp8 = mybir.dt.float8e4
    nc = bass.Bass("TRN2", target_bir_lowering=False, debug=False)
    d = {}
    dts = {"x8": fp8, "hf": f32, "h8": fp8, "w1t8": fp8, "w2t8": fp8,
           "w3t8": fp8, "goutp": f32, "beoutp": f32,
           "b1cp": f32, "g1p": f32, "be1p": f32, "b2p": f32, "b3row": bf16}
    for name, shape in shapes.items():
        d[name] = nc.dram_tensor(name, list(shape), dts[name],
                                 kind="ExternalInput")[:]
    d["out"] = nc.dram_tensor("out", [P, 2, BC], f32, kind="ExternalOutput")[:]
    with tile.TileContext(nc) as tc:
        _emit(nc, tc, d, BC, NE, hsc, flags)
    return nc


def _fold_w_dr(W):
    """W [M, K] -> DoubleRow lhsT tiles [128, K/256, 2, M]:
    [kp, pr, i, j] = W[j, (2*pr+i)*128 + kp]."""
    M, K = W.shape
    return np.ascontiguousarray(
        W.T.reshape(K // 256, 2, P, M).transpose(2, 0, 1, 3))


def _fold_v(v):
    """v [F] -> [128, F/128] with [p, c] = v[c*128+p]."""
    return np.ascontiguousarray(v.reshape(-1, P).T)


def _fold_bm(a):
    """a [N, F] batch-major -> feature-major [128, F/128, N]."""
    n, f = a.shape
    return np.ascontiguousarray(a.T.reshape(f // P, P, n).transpose(1, 0, 2))


def _to_fp8(a):
    import ml_dtypes
    return np.clip(a, -240.0, 240.0).astype(ml_dtypes.float8_e4m3)


def prepare(W1, b1, g1, be1, W2, b2, W3, b3, wt, bt, g_out, be_out, S):
    f32 = np.float32
    const = {}
    W1c = (W1 - W1.mean(axis=0, keepdims=True)) * A1
    const["w1t8"] = _to_fp8(_fold_w_dr(W1c))
    const["w2t8"] = _to_fp8(_fold_w_dr(W2 * A2))
    ts_ = np.linspace(0.0, 1.0, S).astype(f32)
    dt = ts_[1] - ts_[0]
    tm = np.tanh(ts_[: S - 1, None] * wt[None, :].astype(f32)
                 + bt[None, :].astype(f32)).astype(f32)     # [S-1, H]
    steps = [s for s in range(S - 1) if float(np.abs(tm[s]).max()) > 0.0]
    w3s = []
    hsc = []
    for s in steps:
        W3tm = W3 * tm[s][:, None]
        a3 = float(2.0 ** np.floor(np.log2(240.0 / np.abs(W3tm).max())))
        w3s.append(_fold_w_dr(W3tm * a3))
        hsc.append(float(dt) / a3)
    const["w3t8"] = _to_fp8(np.stack(w3s, axis=1))  # [128, NE, 2, 2, 256]
    flags = {
        "has_b1": bool(np.any(b1)),
        "has_aff1": bool(np.any(be1)) or bool(np.any(g1 != 1.0)),
        "has_b2": bool(np.any(b2)),
        "has_b3": bool(np.any(b3)),
        "has_affo": bool(np.any(be_out)) or bool(np.any(g_out != 1.0)),
    }
    if flags["has_b1"]:
        b1c = (b1 - b1.mean()) * A1
        const["b1cp"] = _fold_v(b1c.astype(f32))
    if flags["has_aff1"]:
        const["g1p"] = _fold_v(g1.astype(f32))
        const["be1p"] = _fold_v(be1.astype(f32))
    if flags["has_b2"]:
        const["b2p"] = _fold_v(b2.astype(f32))
    if flags["has_b3"]:
        import ml_dtypes
        const["b3row"] = np.ascontiguousarray(
            b3.astype(f32)[None, :]).astype(ml_dtypes.bfloat16)
    if flags["has_affo"]:
        const["goutp"] = _fold_v(g_out.astype(f32))
        const["beoutp"] = _fold_v(be_out.astype(f32))
    return const, flags, len(steps), tuple(hsc)


def shard_inputs(x, h, ncores):
    B = x.shape[0]
    BC = B // ncores
    per_core = []
    for i in range(ncores):
        sl = slice(i * BC, (i + 1) * BC)
        xc = _fold_bm(np.asarray(x[sl], dtype=np.float32))
        hc = _fold_bm(np.asarray(h[sl], dtype=np.float32))
        per_core.append({
            "x8": _to_fp8(xc),
            "hf": hc,
            "h8": _to_fp8(hc),
        })
    return per_core, BC


def _split_waits_bir(bir_bytes):
    """This container's walrus build rejects >1 embedded sync-wait per
    instruction (and any wait on a Drain). Spill surplus waits into bare
    EventSemaphore instructions just before the owner -- engine program order
    makes the ordering semantics identical, walrus encodes each happily."""
    import json
    m = json.loads(bir_bytes)
    cnt = 0
    for fn in m.get("functions", []):
        for blk in fn.get("blocks", []):
            out = []
            for inst in blk.get("instructions", []):
                body = inst
                si = body.get("sync_info")
                opcode = body.get("opcode")
                waits = (si or {}).get("on_wait") or []
                keep = 0 if opcode == "Drain" else 1
                if si and len(waits) > keep:
                    nspill = len(waits) - keep
                    for w in waits[:nspill]:
                        cnt += 1
                        out.append({
                            "name": f"WSPLIT-{cnt}",
                            "engine": body["engine"],
                            "opcode": "EventSemaphore",
                            "ins": [],
                            "outs": [],
                            "sync_info": {"on_wait": [w], "on_update": []},
                        })
                    si["on_wait"] = waits[nspill:]
                out.append(inst)
            blk["instructions"] = out
    return json.dumps(m).encode()


_BIR_FIX_DONE = False


def _install_bir_fix():
    global _BIR_FIX_DONE
    if _BIR_FIX_DONE:
        return
    import sys
    from concourse import bass_utils as bu

    orig = bu.compile_bir_kernel

    def wrapped(bir_json, tmpdir, neff_name="file.neff"):
        if isinstance(bir_json, str):
            bir_json = bir_json.encode()
        return orig(_split_waits_bir(bir_json), tmpdir, neff_name)

    bu.compile_bir_kernel = wrapped
    b2j = sys.modules.get("concourse.bass2jax")
    if b2j is not None and getattr(b2j, "compile_bir_kernel", None) is orig:
        b2j.compile_bir_kernel = wrapped

    if LDW_OPT:
        orig_run = bu.run_command

        def run2(cmd, **kw):
            if isinstance(cmd, list):
                cmd = ["--enable-ldw-opt=true"
                       if c == "--enable-ldw-opt=false" else c for c in cmd]
            return orig_run(cmd, **kw)

        bu.run_command = run2
    _BIR_FIX_DONE = True


_NC_CACHE = {}
TRACE = False          # set by test.py to capture an NTFF profile
LAST_RESULTS = None    # BassKernelResults of the most recent run


def kernel(x, h, W1, b1, g1, be1, W2, b2, W3, b3, wt, bt, g_out, be_out,
           adapt_steps):
    _install_bir_fix()
    from concourse import bass_utils

    S = int(adapt_steps)
    x = np.asarray(x)
    h = np.asarray(h)
    B, H = h.shape
    assert B % NCORES == 0

    f32 = np.float32
    tm = np.tanh(np.linspace(0.0, 1.0, S)[: S - 1, None].astype(f32)
                 * np.asarray(wt, f32)[None, :] + np.asarray(bt, f32)[None, :])
    if not np.any(tm):
        # every Euler step is multiplied by tm==0: output is just LN(h)
        hv = np.asarray(h, f32)
        mu = hv.mean(-1, keepdims=True)
        var = ((hv - mu) ** 2).mean(-1, keepdims=True)
        y = (hv - mu) / np.sqrt(var + LN_EPS)
        return (y * np.asarray(g_out, f32) + np.asarray(be_out, f32)
                ).astype(np.float32)

    const, flags, NE, hsc = prepare(
        np.asarray(W1, f32), np.asarray(b1, f32), np.asarray(g1, f32),
        np.asarray(be1, f32), np.asarray(W2, f32), np.asarray(b2, f32),
        np.asarray(W3, f32), np.asarray(b3, f32), np.asarray(wt, f32),
        np.asarray(bt, f32), np.asarray(g_out, f32), np.asarray(be_out, f32),
        S)

    shards, BC = shard_inputs(x, h, NCORES)

    key = (BC, NE, hsc, tuple(sorted(flags.items())))
    if key not in _NC_CACHE:
        shapes = {k: v.shape for k, v in const.items()}
        shapes.update({"x8": (P, 2, BC), "hf": (P, 2, BC), "h8": (P, 2, BC)})
        _NC_CACHE[key] = build_nc(BC, NE, hsc, flags, shapes)
    nc = _NC_CACHE[key]

    in_maps = [{**const, **sh} for sh in shards]
    res = bass_utils.run_bass_kernel_spmd(nc, in_maps,
                                          core_ids=list(range(NCORES)),
                                          trace=TRACE)
    global LAST_RESULTS
    LAST_RESULTS = res
    out = np.empty((B, H), dtype=np.float32)
    for i in range(NCORES):
        oc = res.results[i]["out"]                      # [128, 2, BC]
        out[i * BC:(i + 1) * BC] = oc.transpose(2, 1, 0).reshape(BC, H)
    return out


# revision 21
# speedup vs baseline: 1.0274x; 1.0274x over previous
"""Trainium2 Bass kernel v3: LiquidCell (Euler scan over 3-layer MLP+LN).

Changes vs v2 (4.10 ms):
- tm(t=0)=tanh(0)=0: the first Euler step contributes nothing -> run only
  the 8 steps with tm != 0 (host detects zero steps generically).
- Stale-R LayerNorm: normalize each step's L1 PSUM output directly with the
  previous step's 1/sigma (one fused DVE multiply per PSUM group replaces the
  drain + separate normalize passes), then refresh R afterward with the same
  self-correcting Newton form R' = R*(1.5 - 0.5*E[(z*R)^2]).  numpy sim of
  the full pipeline: rel_l2 1.2e-3 (v2 measured 2.2e-3).
- tanh(t*wt)*dt folded into per-step fp8 W3 copies -> h-update is one merged
  scalar_tensor_tensor with an immediate scale; no tmdtp tensor.
- Bigger instructions: one N=4096 gelu for L1, two N=1024 gelus for L2,
  N=1024 fused drains / h-update (PSUM tiles span 2 banks) to amortize the
  ~352-cycle ACT and ~130-cycle DVE fixed costs.
- Variance matmul: sampled squares stored fp8 (scale 1/16) so the reduction
  is a single DoubleRow fp8 matmul.
- Engine balance per tile-step: PE ~8.0us, DVE ~7.2us, ACT ~6.7us,
  GPSIMD ~5.6us.
"""

import numpy as np

P = 128
NCORES = 8
BT = 512          # batch tile (matmul free dim)
G = 4             # tiles interleaved per loop body
LN_EPS = 1e-5
A1 = 16.0         # host scale folded into W1 (cancels in LN)
A2 = 16.0         # host scale folded into W2 (divided out in gelu2 scale)
SS_CH = 2         # z1 chunks sampled for the LN variance estimate (of 8)
LDW_OPT = False   # --enable-ldw-opt=true crashes walrus codegen (CoreV3GenImpl)


def _emit(nc, tc, d, BC, NE, hsc, flags):
    import concourse.mybir as mybir
    from concourse.bass import ds, ts
    from contextlib import ExitStack

    f32 = mybir.dt.float32
    bf16 = mybir.dt.bfloat16
    fp8 = mybir.dt.float8e4
    AF = mybir.ActivationFunctionType
    OP = mybir.AluOpType
    ET = mybir.EngineType
    DR = mybir.MatmulPerfMode.DoubleRow

    has_b1 = flags["has_b1"]
    has_aff1 = flags["has_aff1"]
    has_b2 = flags["has_b2"]
    has_b3 = flags["has_b3"]
    has_affo = flags["has_affo"]

    n_m1 = 8                  # 4H/P
    n_gr = 4                  # L1 PSUM groups of 2 chunks
    n_m2 = 4                  # 2H/P
    n_c = 2                   # H/P

    with ExitStack() as ctx:
        singles = ctx.enter_context(tc.tile_pool(name="singles", bufs=1))
        pool_io = ctx.enter_context(tc.tile_pool(name="io", bufs=2 * G))
        pool_big = ctx.enter_context(tc.tile_pool(name="big", bufs=G))
        pool_w = ctx.enter_context(tc.tile_pool(name="wrk", bufs=4))
        pool_fin = ctx.enter_context(tc.tile_pool(name="fin", bufs=2))
        pool_row = ctx.enter_context(tc.tile_pool(name="rows", bufs=2))
        psA = ctx.enter_context(tc.tile_pool(name="psA", bufs=2, space="PSUM"))
        psB = ctx.enter_context(tc.tile_pool(name="psB", bufs=2, space="PSUM"))

        def load(name, dtype):
            t = singles.tile(list(d[name].shape), dtype, tag=name)
            nc.sync.dma_start(out=t, in_=d[name][:])
            return t

        w1t8 = load("w1t8", fp8)      # [128, 2, 2, 1024]
        w2t8 = load("w2t8", fp8)      # [128, 4, 2, 512]
        w3t8 = load("w3t8", fp8)      # [128, NE, 2, 2, 256]
        b1cp = load("b1cp", f32) if has_b1 else None
        g1p = load("g1p", f32) if has_aff1 else None
        be1p = load("be1p", f32) if has_aff1 else None
        b2p = load("b2p", f32) if has_b2 else None
        b3row = load("b3row", bf16) if has_b3 else None
        goutp = load("goutp", f32) if has_affo else None
        beoutp = load("beoutp", f32) if has_affo else None

        # seed-step variance row: vrow = A1^2 * sigma^2 over SS_CH*P features
        ones_ss = singles.tile([P, 1], bf16)
        nc.vector.memset(ones_ss, 1.0 / (SS_CH * P))
        # steady-state: q01 holds u^2/16; allones8 (fp8 DR lhsT) of 1/16 makes
        # bcv[p,j] = (1/256) sum_{256} u^2 = vhat (the 16s cancel)
        allones8 = singles.tile([P, 2, P], fp8)
        nc.vector.memset(allones8, 1.0 / 16.0)
        onerow = singles.tile([1, P], bf16)
        nc.vector.memset(onerow, 1.0)
        ones_hb = singles.tile([P, 1], bf16)
        nc.vector.memset(ones_hb, 1.0 / (n_c * P))
        eps_t = singles.tile([1, 1], f32)
        nc.vector.memset(eps_t, LN_EPS * A1 * A1)
        eps_o = singles.tile([1, 1], f32)
        nc.vector.memset(eps_o, LN_EPS)
        c15 = singles.tile([P, 1], f32)
        nc.vector.memset(c15, 1.5)
        if has_b3:
            onesrow_bf = singles.tile([1, BT], bf16)
            nc.vector.memset(onesrow_bf, 1.0)

        class T:
            pass

        def make_tile(off, j):
            t = T()
            t.off = off
            t.j = j
            t.x8 = pool_io.tile([P, 2, BT], fp8, tag="x8")
            t.hT = pool_io.tile([P, 2, BT], f32, tag="hT")
            t.h8 = pool_io.tile([P, 2, BT], fp8, tag="h8")
            t.u = pool_big.tile([P, n_m1, BT], bf16, tag="u")
            t.z1g = pool_big.tile([P, n_m1, BT], fp8, tag="z1g")
            t.z2 = pool_big.tile([P, n_m2, BT], fp8, tag="z2")
            t.q01 = pool_big.tile([P, 2, BT], fp8, tag="q01")
            t.rsB = pool_big.tile([P, BT], bf16, tag="rsB")
            nc.sync.dma_start(out=t.x8, in_=d["x8"][:, :, ds(off, BT)])
            nc.sync.dma_start(out=t.hT, in_=d["hf"][:, :, ds(off, BT)])
            nc.sync.dma_start(out=t.h8, in_=d["h8"][:, :, ds(off, BT)])
            return t

        def l1_mms(t, p):
            # one PSUM group = out-chunks 2p, 2p+1; h-pass + x-pass per chunk
            zps = psA.tile([P, 2, BT], f32, tag="l1")
            for c in range(2):
                m = 2 * p + c
                nc.tensor.matmul(zps[:, c, :], lhsT=w1t8[:, 0, :, ts(m, P)],
                                 rhs=t.h8, start=True, stop=False, perf_mode=DR)
                nc.tensor.matmul(zps[:, c, :], lhsT=w1t8[:, 1, :, ts(m, P)],
                                 rhs=t.x8, start=False, stop=True, perf_mode=DR)
            return zps

        def fused_drain(t, p, zps):
            # u = (z + b1) * R  with R from the previous step (stale)
            sl = slice(2 * p, 2 * p + 2)
            rb = t.rsB[:, None, :].to_broadcast((P, 2, BT))
            if has_b1:
                for c in range(2):
                    m = 2 * p + c
                    nc.vector.scalar_tensor_tensor(
                        out=t.u[:, m, :], in0=zps[:, c, :],
                        scalar=b1cp[:, m:m + 1], in1=t.rsB,
                        op0=OP.add, op1=OP.mult)
            else:
                nc.vector.tensor_mul(out=t.u[:, sl, :], in0=zps, in1=rb)

        def sq(t):
            # q01 = u^2/16 (fp8); sampled squares for the variance estimate
            nc.vector.scalar_tensor_tensor(
                out=t.q01, in0=t.u[:, 0:2, :], scalar=1.0 / 16.0,
                in1=t.u[:, 0:2, :], op0=OP.mult, op1=OP.mult)

        def pWB(t):
            # emitted one slot after sq: bcv never blocks the PE FIFO and
            # w is the first ACT op of the slot
            bcv = psB.tile([P, 2, BT], f32, tag="ps2")
            nc.tensor.matmul(bcv[:, 0, :], lhsT=allones8, rhs=t.q01,
                             start=True, stop=True, perf_mode=DR)
            # w = 1.5 - 0.5*vhat ; R <- R*w   (one-Newton rsqrt at seed 1)
            t.wP = pool_w.tile([P, BT], bf16, tag="wP")
            nc.scalar.activation(out=t.wP, in_=bcv[:, 0, :],
                                 func=AF.Identity, bias=c15[:, 0:1],
                                 scale=-0.5)

        def r_mul(t):
            nc.gpsimd.tensor_mul(out=t.rsB, in0=t.rsB, in1=t.wP)

        def gelu1(t):
            if has_aff1:
                for m in range(n_m1):
                    nc.scalar.activation(out=t.z1g[:, m, :], in_=t.u[:, m, :],
                                         func=AF.Gelu, bias=be1p[:, m:m + 1],
                                         scale=g1p[:, m:m + 1])
            else:
                nc.scalar.activation(out=t.z1g, in_=t.u, func=AF.Gelu,
                                     bias=0.0, scale=1.0)

        def pA_seed(t):
            # first real step: raw drains, exact ln/exp 1/sigma, normalize
            for p in range(n_gr):
                zps = l1_mms(t, p)
                if p % 2 == 0:
                    nc.scalar.copy(out=t.u[:, 2 * p:2 * p + 2, :], in_=zps)
                else:
                    nc.vector.tensor_copy(out=t.u[:, 2 * p:2 * p + 2, :],
                                          in_=zps)
            q01b = pool_fin.tile([P, 2, BT], bf16, tag="q0b")
            nc.vector.tensor_mul(out=q01b, in0=t.u[:, 0:2, :],
                                 in1=t.u[:, 0:2, :])
            vps = psB.tile([P, 2, BT], f32, tag="ps2")
            for m in range(SS_CH):
                nc.tensor.matmul(vps[0:1, 0, :], lhsT=ones_ss,
                                 rhs=q01b[:, m, :], start=(m == 0),
                                 stop=(m == SS_CH - 1))
            lnv = pool_row.tile([1, BT], f32, tag="lnv")
            nc.scalar.activation(out=lnv, in_=vps[0:1, 0, :], func=AF.Ln,
                                 bias=eps_t[0:1, 0:1], scale=1.0)
            rsr = pool_row.tile([1, BT], bf16, tag="rsr")
            nc.scalar.activation(out=rsr, in_=lnv, func=AF.Exp,
                                 bias=0.0, scale=-0.5)
            bc = psB.tile([P, 2, BT], f32, tag="ps2")
            nc.tensor.matmul(bc[:, 0, :], lhsT=onerow, rhs=rsr,
                             start=True, stop=True)
            nc.vector.tensor_copy(out=t.rsB, in_=bc[:, 0, :])
            rb = t.rsB[:, None, :].to_broadcast((P, 4, BT))
            for hf in range(2):
                hs = slice(4 * hf, 4 * hf + 4)
                if has_b1:
                    for m in range(4 * hf, 4 * hf + 4):
                        nc.vector.scalar_tensor_tensor(
                            out=t.u[:, m, :], in0=t.u[:, m, :],
                            scalar=b1cp[:, m:m + 1], in1=t.rsB,
                            op0=OP.add, op1=OP.mult)
                else:
                    nc.vector.tensor_mul(out=t.u[:, hs, :], in0=t.u[:, hs, :],
                                         in1=rb)
            gelu1(t)

        def gelu1_half(t, hf):
            hs = slice(4 * hf, 4 * hf + 4)
            if has_aff1:
                for m in range(4 * hf, 4 * hf + 4):
                    nc.scalar.activation(out=t.z1g[:, m, :], in_=t.u[:, m, :],
                                         func=AF.Gelu, bias=be1p[:, m:m + 1],
                                         scale=g1p[:, m:m + 1])
            else:
                nc.scalar.activation(out=t.z1g[:, hs, :], in_=t.u[:, hs, :],
                                     func=AF.Gelu, bias=0.0, scale=1.0)

        def pA(t, seed, last):
            if seed:
                pA_seed(t)
                return
            # gelu1 in halves so z1g completes ~1.7us after the last drain
            # instead of ~3.7us; PE/DVE emission order is untouched
            for p in range(n_gr):
                zps = l1_mms(t, p)
                fused_drain(t, p, zps)
                if p == 0 and not last:
                    sq(t)
                if p == 1:
                    gelu1_half(t, 0)
            gelu1_half(t, 1)

        def pBC(t, i, seed, last):
            # L2 in two halves, straight from PSUM through gelu2
            for h in range(2):
                zps = psB.tile([P, 2, BT], f32, tag="ps2")
                for c in range(2):
                    m = 2 * h + c
                    for p8 in range(4):
                        nc.tensor.matmul(zps[:, c, :],
                                         lhsT=w2t8[:, p8, :, ts(m, P)],
                                         rhs=t.z1g[:, 2 * p8:2 * p8 + 2, :],
                                         start=(p8 == 0), stop=(p8 == 3),
                                         perf_mode=DR)
                if has_b2:
                    for c in range(2):
                        m = 2 * h + c
                        nc.scalar.activation(out=t.z2[:, m, :],
                                             in_=zps[:, c, :], func=AF.Gelu,
                                             bias=b2p[:, m:m + 1],
                                             scale=1.0 / A2)
                else:
                    nc.scalar.activation(out=t.z2[:, 2 * h:2 * h + 2, :],
                                         in_=zps, func=AF.Gelu,
                                         bias=0.0, scale=1.0 / A2)
            # L3 with per-step W3*tm*dt folded weights
            dps = psB.tile([P, 2, BT], f32, tag="ps2")
            for c in range(n_c):
                for p4 in range(2):
                    nc.tensor.matmul(dps[:, c, :],
                                     lhsT=w3t8[:, i, p4, :, ts(c, P)],
                                     rhs=t.z2[:, 2 * p4:2 * p4 + 2, :],
                                     start=(p4 == 0),
                                     stop=(p4 == 1) and not has_b3,
                                     perf_mode=DR)
                if has_b3:
                    nc.tensor.matmul(dps[:, c, :], lhsT=b3row[0:1, ts(c, P)],
                                     rhs=onesrow_bf, start=False, stop=True)
            nc.vector.scalar_tensor_tensor(
                out=t.hT, in0=dps, scalar=hsc[i], in1=t.hT,
                op0=OP.mult, op1=OP.add)
            if not last:
                nc.gpsimd.tensor_copy(out=t.h8, in_=t.hT)
                if not seed:
                    r_mul(t)

        def pfinal(t):
            hb = pool_fin.tile([P, 2, BT], bf16, tag="hb")
            nc.gpsimd.tensor_copy(out=hb, in_=t.hT)
            q2 = pool_fin.tile([P, 2, BT], bf16, tag="q2")
            nc.vector.tensor_mul(out=q2, in0=hb, in1=hb)
            rows = psB.tile([P, 2, BT], f32, tag="ps2")
            for c in range(n_c):
                nc.tensor.matmul(rows[0:1, 0, :], lhsT=ones_hb,
                                 rhs=hb[:, c, :], start=(c == 0),
                                 stop=(c == n_c - 1))
            for c in range(n_c):
                nc.tensor.matmul(rows[0:1, 1, :], lhsT=ones_hb,
                                 rhs=q2[:, c, :], start=(c == 0),
                                 stop=(c == n_c - 1))
            mu2 = pool_row.tile([1, BT], f32, tag="mu2")
            nc.vector.tensor_copy(out=mu2, in_=rows[0:1, 0, :])
            var2 = pool_row.tile([1, BT], f32, tag="var2")
            msq = pool_row.tile([1, BT], f32, tag="msq")
            nc.vector.tensor_mul(out=msq, in0=mu2, in1=mu2)
            nc.vector.tensor_sub(out=var2, in0=rows[0:1, 1, :], in1=msq)
            lnv2 = pool_row.tile([1, BT], f32, tag="lnv2")
            nc.scalar.activation(out=lnv2, in_=var2, func=AF.Ln,
                                 bias=eps_o[0:1, 0:1], scale=1.0)
            rs2 = pool_row.tile([1, BT], bf16, tag="rs2")
            nc.scalar.activation(out=rs2, in_=lnv2, func=AF.Exp,
                                 bias=0.0, scale=-0.5)
            mu2b = pool_row.tile([1, BT], bf16, tag="mu2b")
            nc.vector.tensor_copy(out=mu2b, in_=mu2)
            bmrs = psB.tile([P, 2, BT], f32, tag="ps2")
            nc.tensor.matmul(bmrs[:, 0, :], lhsT=onerow, rhs=mu2b,
                             start=True, stop=True)
            nc.tensor.matmul(bmrs[:, 1, :], lhsT=onerow, rhs=rs2,
                             start=True, stop=True)
            dd = pool_fin.tile([P, 2, BT], f32, tag="dd")
            nc.vector.tensor_sub(
                out=dd, in0=t.hT,
                in1=bmrs[:, 0:1, :].to_broadcast((P, 2, BT)))
            nc.vector.tensor_mul(
                out=dd, in0=dd,
                in1=bmrs[:, 1:2, :].to_broadcast((P, 2, BT)))
            if has_affo:
                oc = pool_fin.tile([P, 2, BT], f32, tag="oc")
                for c in range(n_c):
                    nc.scalar.activation(out=oc[:, c, :], in_=dd[:, c, :],
                                         func=AF.Identity,
                                         bias=beoutp[:, c:c + 1],
                                         scale=goutp[:, c:c + 1])
                dd = oc
            nc.scalar.dma_start(out=d["out"][:, :, ds(t.off, BT)], in_=dd)

        REP = 2 if BC % (2 * G * BT) == 0 else 1
        assert BC % (REP * G * BT) == 0
        with tc.For_i(0, BC, REP * G * BT,
                      hint_engines=(ET.PE, ET.Activation, ET.DVE)) as i0:
            # flat pipeline over REP*NE*G tile-slots: pBC lags pA by 2 slots,
            # pWB by 1, so each step's L2/L3 tail interleaves with the next
            # step's L1 and the h-update STTs never sit ahead of the next
            # drains in DVE's FIFO.  REP=2 halves the loop-wrap bubble.
            slots = [(r, i, j) for r in range(REP)
                     for i in range(NE) for j in range(G)]
            tiles = {}
            for k in range(len(slots) + 2):
                if k < len(slots):
                    r, i, j = slots[k]
                    if i == 0:
                        tiles[(r, j)] = make_tile(
                            i0 if (r, j) == (0, 0)
                            else i0 + (r * G + j) * BT, j)
                    pA(tiles[(r, j)], i == 0, i == NE - 1)
                if 1 <= k <= len(slots):
                    r, i, j = slots[k - 1]
                    if 0 < i < NE - 1:
                        pWB(tiles[(r, j)])
                if k >= 2:
                    r, i, j = slots[k - 2]
                    pBC(tiles[(r, j)], i, i == 0, i == NE - 1)
                    if i == NE - 1:
                        pfinal(tiles[(r, j)])


def build_nc(BC, NE, hsc, flags, shapes):
    import concourse.bass as bass
    import concourse.mybir as mybir
    import concourse.tile as tile

    f32 = mybir.dt.float32
    bf16 = mybir.dt.bfloat16
    fp8 = mybir.dt.float8e4
    nc = bass.Bass("TRN2", target_bir_lowering=False, debug=False)
    d = {}
    dts = {"x8": fp8, "hf": f32, "h8": fp8, "w1t8": fp8, "w2t8": fp8,
           "w3t8": fp8, "goutp": f32, "beoutp": f32,
           "b1cp": f32, "g1p": f32, "be1p": f32, "b2p": f32, "b3row": bf16}
    for name, shape in shapes.items():
        d[name] = nc.dram_tensor(name, list(shape), dts[name],
                                 kind="ExternalInput")[:]
    d["out"] = nc.dram_tensor("out", [P, 2, BC], f32, kind="ExternalOutput")[:]
    with tile.TileContext(nc) as tc:
        _emit(nc, tc, d, BC, NE, hsc, flags)
    return nc


def _fold_w_dr(W):
    """W [M, K] -> DoubleRow lhsT tiles [128, K/256, 2, M]:
    [kp, pr, i, j] = W[j, (2*pr+i)*128 + kp]."""
    M, K = W.shape
    return np.ascontiguousarray(
        W.T.reshape(K // 256, 2, P, M).transpose(2, 0, 1, 3))


def _fold_v(v):
    """v [F] -> [128, F/128] with [p, c] = v[c*128+p]."""
    return np.ascontiguousarray(v.reshape(-1, P).T)


def _fold_bm(a):
    """a [N, F] batch-major -> feature-major [128, F/128, N]."""
    n, f = a.shape
    return np.ascontiguousarray(a.T.reshape(f // P, P, n).transpose(1, 0, 2))


def _to_fp8(a):
    import ml_dtypes
    return np.clip(a, -240.0, 240.0).astype(ml_dtypes.float8_e4m3)


def prepare(W1, b1, g1, be1, W2, b2, W3, b3, wt, bt, g_out, be_out, S):
    f32 = np.float32
    const = {}
    W1c = (W1 - W1.mean(axis=0, keepdims=True)) * A1
    const["w1t8"] = _to_fp8(_fold_w_dr(W1c))
    const["w2t8"] = _to_fp8(_fold_w_dr(W2 * A2))
    ts_ = np.linspace(0.0, 1.0, S).astype(f32)
    dt = ts_[1] - ts_[0]
    tm = np.tanh(ts_[: S - 1, None] * wt[None, :].astype(f32)
                 + bt[None, :].astype(f32)).astype(f32)     # [S-1, H]
    steps = [s for s in range(S - 1) if float(np.abs(tm[s]).max()) > 0.0]
    w3s = []
    hsc = []
    for s in steps:
        W3tm = W3 * tm[s][:, None]
        a3 = float(2.0 ** np.floor(np.log2(240.0 / np.abs(W3tm).max())))
        w3s.append(_fold_w_dr(W3tm * a3))
        hsc.append(float(dt) / a3)
    const["w3t8"] = _to_fp8(np.stack(w3s, axis=1))  # [128, NE, 2, 2, 256]
    flags = {
        "has_b1": bool(np.any(b1)),
        "has_aff1": bool(np.any(be1)) or bool(np.any(g1 != 1.0)),
        "has_b2": bool(np.any(b2)),
        "has_b3": bool(np.any(b3)),
        "has_affo": bool(np.any(be_out)) or bool(np.any(g_out != 1.0)),
    }
    if flags["has_b1"]:
        b1c = (b1 - b1.mean()) * A1
        const["b1cp"] = _fold_v(b1c.astype(f32))
    if flags["has_aff1"]:
        const["g1p"] = _fold_v(g1.astype(f32))
        const["be1p"] = _fold_v(be1.astype(f32))
    if flags["has_b2"]:
        const["b2p"] = _fold_v(b2.astype(f32))
    if flags["has_b3"]:
        import ml_dtypes
        const["b3row"] = np.ascontiguousarray(
            b3.astype(f32)[None, :]).astype(ml_dtypes.bfloat16)
    if flags["has_affo"]:
        const["goutp"] = _fold_v(g_out.astype(f32))
        const["beoutp"] = _fold_v(be_out.astype(f32))
    return const, flags, len(steps), tuple(hsc)


def shard_inputs(x, h, ncores):
    B = x.shape[0]
    BC = B // ncores
    per_core = []
    for i in range(ncores):
        sl = slice(i * BC, (i + 1) * BC)
        xc = _fold_bm(np.asarray(x[sl], dtype=np.float32))
        hc = _fold_bm(np.asarray(h[sl], dtype=np.float32))
        per_core.append({
            "x8": _to_fp8(xc),
            "hf": hc,
            "h8": _to_fp8(hc),
        })
    return per_core, BC


def _split_waits_bir(bir_bytes):
    """This container's walrus build rejects >1 embedded sync-wait per
    instruction (and any wait on a Drain). Spill surplus waits into bare
    EventSemaphore instructions just before the owner -- engine program order
    makes the ordering semantics identical, walrus encodes each happily."""
    import json
    m = json.loads(bir_bytes)
    cnt = 0
    for fn in m.get("functions", []):
        for blk in fn.get("blocks", []):
            out = []
            for inst in blk.get("instructions", []):
                body = inst
                si = body.get("sync_info")
                opcode = body.get("opcode")
                waits = (si or {}).get("on_wait") or []
                keep = 0 if opcode == "Drain" else 1
                if si and len(waits) > keep:
                    nspill = len(waits) - keep
                    for w in waits[:nspill]:
                        cnt += 1
                        out.append({
                            "name": f"WSPLIT-{cnt}",
                            "engine": body["engine"],
                            "opcode": "EventSemaphore",
                            "ins": [],
                            "outs": [],
                            "sync_info": {"on_wait": [w], "on_update": []},
                        })
                    si["on_wait"] = waits[nspill:]
                out.append(inst)
            blk["instructions"] = out
    return json.dumps(m).encode()


_BIR_FIX_DONE = False


def _install_bir_fix():
    global _BIR_FIX_DONE
    if _BIR_FIX_DONE:
        return
    import sys
    from concourse import bass_utils as bu

    orig = bu.compile_bir_kernel

    def wrapped(bir_json, tmpdir, neff_name="file.neff"):
        if isinstance(bir_json, str):
            bir_json = bir_json.encode()
        return orig(_split_waits_bir(bir_json), tmpdir, neff_name)

    bu.compile_bir_kernel = wrapped
    b2j = sys.modules.get("concourse.bass2jax")
    if b2j is not None and getattr(b2j, "compile_bir_kernel", None) is orig:
        b2j.compile_bir_kernel = wrapped

    if LDW_OPT:
        orig_run = bu.run_command

        def run2(cmd, **kw):
            if isinstance(cmd, list):
                cmd = ["--enable-ldw-opt=true"
                       if c == "--enable-ldw-opt=false" else c for c in cmd]
            return orig_run(cmd, **kw)

        bu.run_command = run2
    _BIR_FIX_DONE = True


_NC_CACHE = {}
TRACE = False          # set by test.py to capture an NTFF profile
LAST_RESULTS = None    # BassKernelResults of the most recent run


def kernel(x, h, W1, b1, g1, be1, W2, b2, W3, b3, wt, bt, g_out, be_out,
           adapt_steps):
    _install_bir_fix()
    from concourse import bass_utils

    S = int(adapt_steps)
    x = np.asarray(x)
    h = np.asarray(h)
    B, H = h.shape
    assert B % NCORES == 0

    f32 = np.float32
    tm = np.tanh(np.linspace(0.0, 1.0, S)[: S - 1, None].astype(f32)
                 * np.asarray(wt, f32)[None, :] + np.asarray(bt, f32)[None, :])
    if not np.any(tm):
        # every Euler step is multiplied by tm==0: output is just LN(h)
        hv = np.asarray(h, f32)
        mu = hv.mean(-1, keepdims=True)
        var = ((hv - mu) ** 2).mean(-1, keepdims=True)
        y = (hv - mu) / np.sqrt(var + LN_EPS)
        return (y * np.asarray(g_out, f32) + np.asarray(be_out, f32)
                ).astype(np.float32)

    const, flags, NE, hsc = prepare(
        np.asarray(W1, f32), np.asarray(b1, f32), np.asarray(g1, f32),
        np.asarray(be1, f32), np.asarray(W2, f32), np.asarray(b2, f32),
        np.asarray(W3, f32), np.asarray(b3, f32), np.asarray(wt, f32),
        np.asarray(bt, f32), np.asarray(g_out, f32), np.asarray(be_out, f32),
        S)

    shards, BC = shard_inputs(x, h, NCORES)

    key = (BC, NE, hsc, tuple(sorted(flags.items())))
    if key not in _NC_CACHE:
        shapes = {k: v.shape for k, v in const.items()}
        shapes.update({"x8": (P, 2, BC), "hf": (P, 2, BC), "h8": (P, 2, BC)})
        _NC_CACHE[key] = build_nc(BC, NE, hsc, flags, shapes)
    nc = _NC_CACHE[key]

    in_maps = [{**const, **sh} for sh in shards]
    res = bass_utils.run_bass_kernel_spmd(nc, in_maps,
                                          core_ids=list(range(NCORES)),
                                          trace=TRACE)
    global LAST_RESULTS
    LAST_RESULTS = res
    out = np.empty((B, H), dtype=np.float32)
    for i in range(NCORES):
        oc = res.results[i]["out"]                      # [128, 2, BC]
        out[i * BC:(i + 1) * BC] = oc.transpose(2, 1, 0).reshape(BC, H)
    return out


# revision 22
# speedup vs baseline: 1.0347x; 1.0071x over previous
"""Trainium2 Bass kernel v3: LiquidCell (Euler scan over 3-layer MLP+LN).

Changes vs v2 (4.10 ms):
- tm(t=0)=tanh(0)=0: the first Euler step contributes nothing -> run only
  the 8 steps with tm != 0 (host detects zero steps generically).
- Stale-R LayerNorm: normalize each step's L1 PSUM output directly with the
  previous step's 1/sigma (one fused DVE multiply per PSUM group replaces the
  drain + separate normalize passes), then refresh R afterward with the same
  self-correcting Newton form R' = R*(1.5 - 0.5*E[(z*R)^2]).  numpy sim of
  the full pipeline: rel_l2 1.2e-3 (v2 measured 2.2e-3).
- tanh(t*wt)*dt folded into per-step fp8 W3 copies -> h-update is one merged
  scalar_tensor_tensor with an immediate scale; no tmdtp tensor.
- Bigger instructions: one N=4096 gelu for L1, two N=1024 gelus for L2,
  N=1024 fused drains / h-update (PSUM tiles span 2 banks) to amortize the
  ~352-cycle ACT and ~130-cycle DVE fixed costs.
- Variance matmul: sampled squares stored fp8 (scale 1/16) so the reduction
  is a single DoubleRow fp8 matmul.
- Engine balance per tile-step: PE ~8.0us, DVE ~7.2us, ACT ~6.7us,
  GPSIMD ~5.6us.
"""

import numpy as np

P = 128
NCORES = 8
BT = 512          # batch tile (matmul free dim)
G = 4             # tiles interleaved per loop body
LN_EPS = 1e-5
A1 = 16.0         # host scale folded into W1 (cancels in LN)
A2 = 16.0         # host scale folded into W2 (divided out in gelu2 scale)
SS_CH = 2         # z1 chunks sampled for the LN variance estimate (of 8)
LDW_OPT = False   # --enable-ldw-opt=true crashes walrus codegen (CoreV3GenImpl)


def _emit(nc, tc, d, BC, NE, hsc, flags):
    import concourse.mybir as mybir
    from concourse.bass import ds, ts
    from contextlib import ExitStack

    f32 = mybir.dt.float32
    bf16 = mybir.dt.bfloat16
    fp8 = mybir.dt.float8e4
    AF = mybir.ActivationFunctionType
    OP = mybir.AluOpType
    ET = mybir.EngineType
    DR = mybir.MatmulPerfMode.DoubleRow

    has_b1 = flags["has_b1"]
    has_aff1 = flags["has_aff1"]
    has_b2 = flags["has_b2"]
    has_b3 = flags["has_b3"]
    has_affo = flags["has_affo"]

    n_m1 = 8                  # 4H/P
    n_gr = 4                  # L1 PSUM groups of 2 chunks
    n_m2 = 4                  # 2H/P
    n_c = 2                   # H/P

    with ExitStack() as ctx:
        singles = ctx.enter_context(tc.tile_pool(name="singles", bufs=1))
        pool_io = ctx.enter_context(tc.tile_pool(name="io", bufs=2 * G))
        pool_big = ctx.enter_context(tc.tile_pool(name="big", bufs=G))
        pool_w = ctx.enter_context(tc.tile_pool(name="wrk", bufs=4))
        pool_fin = ctx.enter_context(tc.tile_pool(name="fin", bufs=2))
        pool_row = ctx.enter_context(tc.tile_pool(name="rows", bufs=2))
        psA = ctx.enter_context(tc.tile_pool(name="psA", bufs=2, space="PSUM"))
        psB = ctx.enter_context(tc.tile_pool(name="psB", bufs=2, space="PSUM"))

        def load(name, dtype):
            t = singles.tile(list(d[name].shape), dtype, tag=name)
            nc.sync.dma_start(out=t, in_=d[name][:])
            return t

        w1t8 = load("w1t8", fp8)      # [128, 2, 2, 1024]
        w2t8 = load("w2t8", fp8)      # [128, 4, 2, 512]
        w3t8 = load("w3t8", fp8)      # [128, NE, 2, 2, 256]
        b1cp = load("b1cp", f32) if has_b1 else None
        g1p = load("g1p", f32) if has_aff1 else None
        be1p = load("be1p", f32) if has_aff1 else None
        b2p = load("b2p", f32) if has_b2 else None
        b3row = load("b3row", bf16) if has_b3 else None
        goutp = load("goutp", f32) if has_affo else None
        beoutp = load("beoutp", f32) if has_affo else None

        # seed-step variance row: vrow = A1^2 * sigma^2 over SS_CH*P features
        ones_ss = singles.tile([P, 1], bf16)
        nc.vector.memset(ones_ss, 1.0 / (SS_CH * P))
        # steady-state: q01 holds u^2/16; allones8 (fp8 DR lhsT) of 1/16 makes
        # bcv[p,j] = (1/256) sum_{256} u^2 = vhat (the 16s cancel)
        allones8 = singles.tile([P, 2, P], fp8)
        nc.vector.memset(allones8, 1.0 / 16.0)
        onerow = singles.tile([1, P], bf16)
        nc.vector.memset(onerow, 1.0)
        ones_hb = singles.tile([P, 1], bf16)
        nc.vector.memset(ones_hb, 1.0 / (n_c * P))
        eps_t = singles.tile([1, 1], f32)
        nc.vector.memset(eps_t, LN_EPS * A1 * A1)
        eps_o = singles.tile([1, 1], f32)
        nc.vector.memset(eps_o, LN_EPS)
        c15 = singles.tile([P, 1], f32)
        nc.vector.memset(c15, 1.5)
        dmy = singles.tile([1, 1], f32)
        if has_b3:
            onesrow_bf = singles.tile([1, BT], bf16)
            nc.vector.memset(onesrow_bf, 1.0)

        class T:
            pass

        def make_tile(off, j):
            t = T()
            t.off = off
            t.j = j
            t.x8 = pool_io.tile([P, 2, BT], fp8, tag="x8")
            t.hT = pool_io.tile([P, 2, BT], f32, tag="hT")
            t.h8 = pool_io.tile([P, 2, BT], fp8, tag="h8")
            t.u = pool_big.tile([P, n_m1, BT], bf16, tag="u")
            t.z1g = pool_big.tile([P, n_m1, BT], fp8, tag="z1g")
            t.z2 = pool_big.tile([P, n_m2, BT], fp8, tag="z2")
            t.q01 = pool_big.tile([P, 2, BT], fp8, tag="q01")
            t.rsB = pool_big.tile([P, BT], bf16, tag="rsB")
            nc.sync.dma_start(out=t.x8, in_=d["x8"][:, :, ds(off, BT)])
            nc.sync.dma_start(out=t.hT, in_=d["hf"][:, :, ds(off, BT)])
            nc.sync.dma_start(out=t.h8, in_=d["h8"][:, :, ds(off, BT)])
            return t

        def l1_mms(t, p):
            # one PSUM group = out-chunks 2p, 2p+1; h-pass + x-pass per chunk
            zps = psA.tile([P, 2, BT], f32, tag="l1")
            for c in range(2):
                m = 2 * p + c
                nc.tensor.matmul(zps[:, c, :], lhsT=w1t8[:, 0, :, ts(m, P)],
                                 rhs=t.h8, start=True, stop=False, perf_mode=DR)
                nc.tensor.matmul(zps[:, c, :], lhsT=w1t8[:, 1, :, ts(m, P)],
                                 rhs=t.x8, start=False, stop=True, perf_mode=DR)
            return zps

        def fused_drain(t, p, zps):
            # u = (z + b1) * R  with R from the previous step (stale)
            sl = slice(2 * p, 2 * p + 2)
            rb = t.rsB[:, None, :].to_broadcast((P, 2, BT))
            if has_b1:
                for c in range(2):
                    m = 2 * p + c
                    nc.vector.scalar_tensor_tensor(
                        out=t.u[:, m, :], in0=zps[:, c, :],
                        scalar=b1cp[:, m:m + 1], in1=t.rsB,
                        op0=OP.add, op1=OP.mult)
            else:
                nc.vector.tensor_mul(out=t.u[:, sl, :], in0=zps, in1=rb)

        def sq(t):
            # q01 = u^2/16 (fp8); sampled squares for the variance estimate
            nc.vector.scalar_tensor_tensor(
                out=t.q01, in0=t.u[:, 0:2, :], scalar=1.0 / 16.0,
                in1=t.u[:, 0:2, :], op0=OP.mult, op1=OP.mult)

        def pWB(t):
            # emitted one slot after sq: bcv never blocks the PE FIFO and
            # w is the first ACT op of the slot
            bcv = psB.tile([P, 2, BT], f32, tag="ps2")
            nc.tensor.matmul(bcv[:, 0, :], lhsT=allones8, rhs=t.q01,
                             start=True, stop=True, perf_mode=DR)
            # w = 1.5 - 0.5*vhat ; R <- R*w   (one-Newton rsqrt at seed 1)
            t.wP = pool_w.tile([P, BT], bf16, tag="wP")
            nc.scalar.activation(out=t.wP, in_=bcv[:, 0, :],
                                 func=AF.Identity, bias=c15[:, 0:1],
                                 scale=-0.5)

        def r_mul(t):
            nc.gpsimd.tensor_mul(out=t.rsB, in0=t.rsB, in1=t.wP)

        def gelu1(t):
            if has_aff1:
                for m in range(n_m1):
                    nc.scalar.activation(out=t.z1g[:, m, :], in_=t.u[:, m, :],
                                         func=AF.Gelu, bias=be1p[:, m:m + 1],
                                         scale=g1p[:, m:m + 1])
            else:
                nc.scalar.activation(out=t.z1g, in_=t.u, func=AF.Gelu,
                                     bias=0.0, scale=1.0)

        def pA_seed(t):
            # first real step: raw drains, exact ln/exp 1/sigma, normalize.
            # dummy Ln up front: the ACT table load runs during the L1
            # matmuls instead of sitting on the serial stats chain
            nc.scalar.activation(out=dmy, in_=eps_o[0:1, 0:1], func=AF.Ln,
                                 bias=0.0, scale=1.0)
            for p in range(n_gr):
                zps = l1_mms(t, p)
                if p % 2 == 0:
                    nc.scalar.copy(out=t.u[:, 2 * p:2 * p + 2, :], in_=zps)
                else:
                    nc.vector.tensor_copy(out=t.u[:, 2 * p:2 * p + 2, :],
                                          in_=zps)
            q01b = pool_fin.tile([P, 2, BT], bf16, tag="q0b")
            nc.vector.tensor_mul(out=q01b, in0=t.u[:, 0:2, :],
                                 in1=t.u[:, 0:2, :])
            vps = psB.tile([P, 2, BT], f32, tag="ps2")
            for m in range(SS_CH):
                nc.tensor.matmul(vps[0:1, 0, :], lhsT=ones_ss,
                                 rhs=q01b[:, m, :], start=(m == 0),
                                 stop=(m == SS_CH - 1))
            lnv = pool_row.tile([1, BT], f32, tag="lnv")
            nc.scalar.activation(out=lnv, in_=vps[0:1, 0, :], func=AF.Ln,
                                 bias=eps_t[0:1, 0:1], scale=1.0)
            rsr = pool_row.tile([1, BT], bf16, tag="rsr")
            nc.scalar.activation(out=rsr, in_=lnv, func=AF.Exp,
                                 bias=0.0, scale=-0.5)
            bc = psB.tile([P, 2, BT], f32, tag="ps2")
            nc.tensor.matmul(bc[:, 0, :], lhsT=onerow, rhs=rsr,
                             start=True, stop=True)
            nc.vector.tensor_copy(out=t.rsB, in_=bc[:, 0, :])
            rb = t.rsB[:, None, :].to_broadcast((P, 4, BT))
            for hf in range(2):
                hs = slice(4 * hf, 4 * hf + 4)
                if has_b1:
                    for m in range(4 * hf, 4 * hf + 4):
                        nc.vector.scalar_tensor_tensor(
                            out=t.u[:, m, :], in0=t.u[:, m, :],
                            scalar=b1cp[:, m:m + 1], in1=t.rsB,
                            op0=OP.add, op1=OP.mult)
                else:
                    nc.vector.tensor_mul(out=t.u[:, hs, :], in0=t.u[:, hs, :],
                                         in1=rb)
                # gelu each half right after its normalize: the Gelu table
                # reload hides under the second normalize multiply
                gelu1_half(t, hf)

        def gelu1_half(t, hf):
            hs = slice(4 * hf, 4 * hf + 4)
            if has_aff1:
                for m in range(4 * hf, 4 * hf + 4):
                    nc.scalar.activation(out=t.z1g[:, m, :], in_=t.u[:, m, :],
                                         func=AF.Gelu, bias=be1p[:, m:m + 1],
                                         scale=g1p[:, m:m + 1])
            else:
                nc.scalar.activation(out=t.z1g[:, hs, :], in_=t.u[:, hs, :],
                                     func=AF.Gelu, bias=0.0, scale=1.0)

        def pA(t, seed, last):
            if seed:
                pA_seed(t)
                return
            # gelu1 in halves so z1g completes ~1.7us after the last drain
            # instead of ~3.7us; PE/DVE emission order is untouched
            for p in range(n_gr):
                zps = l1_mms(t, p)
                fused_drain(t, p, zps)
                if p == 0 and not last:
                    sq(t)
                if p == 1:
                    gelu1_half(t, 0)
            gelu1_half(t, 1)

        def pBC(t, i, seed, last):
            # L2 in two halves, straight from PSUM through gelu2
            for h in range(2):
                zps = psB.tile([P, 2, BT], f32, tag="ps2")
                for c in range(2):
                    m = 2 * h + c
                    for p8 in range(4):
                        nc.tensor.matmul(zps[:, c, :],
                                         lhsT=w2t8[:, p8, :, ts(m, P)],
                                         rhs=t.z1g[:, 2 * p8:2 * p8 + 2, :],
                                         start=(p8 == 0), stop=(p8 == 3),
                                         perf_mode=DR)
                if has_b2:
                    for c in range(2):
                        m = 2 * h + c
                        nc.scalar.activation(out=t.z2[:, m, :],
                                             in_=zps[:, c, :], func=AF.Gelu,
                                             bias=b2p[:, m:m + 1],
                                             scale=1.0 / A2)
                else:
                    nc.scalar.activation(out=t.z2[:, 2 * h:2 * h + 2, :],
                                         in_=zps, func=AF.Gelu,
                                         bias=0.0, scale=1.0 / A2)
            # L3 with per-step W3*tm*dt folded weights
            dps = psB.tile([P, 2, BT], f32, tag="ps2")
            for c in range(n_c):
                for p4 in range(2):
                    nc.tensor.matmul(dps[:, c, :],
                                     lhsT=w3t8[:, i, p4, :, ts(c, P)],
                                     rhs=t.z2[:, 2 * p4:2 * p4 + 2, :],
                                     start=(p4 == 0),
                                     stop=(p4 == 1) and not has_b3,
                                     perf_mode=DR)
                if has_b3:
                    nc.tensor.matmul(dps[:, c, :], lhsT=b3row[0:1, ts(c, P)],
                                     rhs=onesrow_bf, start=False, stop=True)
            nc.vector.scalar_tensor_tensor(
                out=t.hT, in0=dps, scalar=hsc[i], in1=t.hT,
                op0=OP.mult, op1=OP.add)
            if not last:
                nc.gpsimd.tensor_copy(out=t.h8, in_=t.hT)
                if not seed:
                    r_mul(t)

        def pfinal(t):
            hb = pool_fin.tile([P, 2, BT], bf16, tag="hb")
            nc.gpsimd.tensor_copy(out=hb, in_=t.hT)
            q2 = pool_fin.tile([P, 2, BT], bf16, tag="q2")
            nc.vector.tensor_mul(out=q2, in0=hb, in1=hb)
            rows = psB.tile([P, 2, BT], f32, tag="ps2")
            for c in range(n_c):
                nc.tensor.matmul(rows[0:1, 0, :], lhsT=ones_hb,
                                 rhs=hb[:, c, :], start=(c == 0),
                                 stop=(c == n_c - 1))
            for c in range(n_c):
                nc.tensor.matmul(rows[0:1, 1, :], lhsT=ones_hb,
                                 rhs=q2[:, c, :], start=(c == 0),
                                 stop=(c == n_c - 1))
            mu2 = pool_row.tile([1, BT], f32, tag="mu2")
            nc.vector.tensor_copy(out=mu2, in_=rows[0:1, 0, :])
            var2 = pool_row.tile([1, BT], f32, tag="var2")
            msq = pool_row.tile([1, BT], f32, tag="msq")
            nc.vector.tensor_mul(out=msq, in0=mu2, in1=mu2)
            nc.vector.tensor_sub(out=var2, in0=rows[0:1, 1, :], in1=msq)
            lnv2 = pool_row.tile([1, BT], f32, tag="lnv2")
            nc.scalar.activation(out=lnv2, in_=var2, func=AF.Ln,
                                 bias=eps_o[0:1, 0:1], scale=1.0)
            rs2 = pool_row.tile([1, BT], bf16, tag="rs2")
            nc.scalar.activation(out=rs2, in_=lnv2, func=AF.Exp,
                                 bias=0.0, scale=-0.5)
            mu2b = pool_row.tile([1, BT], bf16, tag="mu2b")
            nc.vector.tensor_copy(out=mu2b, in_=mu2)
            bmrs = psB.tile([P, 2, BT], f32, tag="ps2")
            nc.tensor.matmul(bmrs[:, 0, :], lhsT=onerow, rhs=mu2b,
                             start=True, stop=True)
            nc.tensor.matmul(bmrs[:, 1, :], lhsT=onerow, rhs=rs2,
                             start=True, stop=True)
            dd = pool_fin.tile([P, 2, BT], f32, tag="dd")
            nc.vector.tensor_sub(
                out=dd, in0=t.hT,
                in1=bmrs[:, 0:1, :].to_broadcast((P, 2, BT)))
            nc.vector.tensor_mul(
                out=dd, in0=dd,
                in1=bmrs[:, 1:2, :].to_broadcast((P, 2, BT)))
            if has_affo:
                oc = pool_fin.tile([P, 2, BT], f32, tag="oc")
                for c in range(n_c):
                    nc.scalar.activation(out=oc[:, c, :], in_=dd[:, c, :],
                                         func=AF.Identity,
                                         bias=beoutp[:, c:c + 1],
                                         scale=goutp[:, c:c + 1])
                dd = oc
            nc.scalar.dma_start(out=d["out"][:, :, ds(t.off, BT)], in_=dd)

        REP = 2 if BC % (2 * G * BT) == 0 else 1
        assert BC % (REP * G * BT) == 0
        with tc.For_i(0, BC, REP * G * BT,
                      hint_engines=(ET.PE, ET.Activation, ET.DVE)) as i0:
            # flat pipeline over REP*NE*G tile-slots: pBC lags pA by 2 slots,
            # pWB by 1, so each step's L2/L3 tail interleaves with the next
            # step's L1 and the h-update STTs never sit ahead of the next
            # drains in DVE's FIFO.  REP=2 halves the loop-wrap bubble.
            slots = [(r, i, j) for r in range(REP)
                     for i in range(NE) for j in range(G)]
            tiles = {}
            for k in range(len(slots) + 2):
                if k < len(slots):
                    r, i, j = slots[k]
                    if i == 0:
                        tiles[(r, j)] = make_tile(
                            i0 if (r, j) == (0, 0)
                            else i0 + (r * G + j) * BT, j)
                    pA(tiles[(r, j)], i == 0, i == NE - 1)
                if 1 <= k <= len(slots):
                    r, i, j = slots[k - 1]
                    if 0 < i < NE - 1:
                        pWB(tiles[(r, j)])
                if k >= 2:
                    r, i, j = slots[k - 2]
                    pBC(tiles[(r, j)], i, i == 0, i == NE - 1)
                    if i == NE - 1:
                        pfinal(tiles[(r, j)])


def build_nc(BC, NE, hsc, flags, shapes):
    import concourse.bass as bass
    import concourse.mybir as mybir
    import concourse.tile as tile

    f32 = mybir.dt.float32
    bf16 = mybir.dt.bfloat16
    fp8 = mybir.dt.float8e4
    nc = bass.Bass("TRN2", target_bir_lowering=False, debug=False)
    d = {}
    dts = {"x8": fp8, "hf": f32, "h8": fp8, "w1t8": fp8, "w2t8": fp8,
           "w3t8": fp8, "goutp": f32, "beoutp": f32,
           "b1cp": f32, "g1p": f32, "be1p": f32, "b2p": f32, "b3row": bf16}
    for name, shape in shapes.items():
        d[name] = nc.dram_tensor(name, list(shape), dts[name],
                                 kind="ExternalInput")[:]
    d["out"] = nc.dram_tensor("out", [P, 2, BC], f32, kind="ExternalOutput")[:]
    with tile.TileContext(nc) as tc:
        _emit(nc, tc, d, BC, NE, hsc, flags)
    return nc


def _fold_w_dr(W):
    """W [M, K] -> DoubleRow lhsT tiles [128, K/256, 2, M]:
    [kp, pr, i, j] = W[j, (2*pr+i)*128 + kp]."""
    M, K = W.shape
    return np.ascontiguousarray(
        W.T.reshape(K // 256, 2, P, M).transpose(2, 0, 1, 3))


def _fold_v(v):
    """v [F] -> [128, F/128] with [p, c] = v[c*128+p]."""
    return np.ascontiguousarray(v.reshape(-1, P).T)


def _fold_bm(a):
    """a [N, F] batch-major -> feature-major [128, F/128, N]."""
    n, f = a.shape
    return np.ascontiguousarray(a.T.reshape(f // P, P, n).transpose(1, 0, 2))


def _to_fp8(a):
    import ml_dtypes
    return np.clip(a, -240.0, 240.0).astype(ml_dtypes.float8_e4m3)


def prepare(W1, b1, g1, be1, W2, b2, W3, b3, wt, bt, g_out, be_out, S):
    f32 = np.float32
    const = {}
    W1c = (W1 - W1.mean(axis=0, keepdims=True)) * A1
    const["w1t8"] = _to_fp8(_fold_w_dr(W1c))
    const["w2t8"] = _to_fp8(_fold_w_dr(W2 * A2))
    ts_ = np.linspace(0.0, 1.0, S).astype(f32)
    dt = ts_[1] - ts_[0]
    tm = np.tanh(ts_[: S - 1, None] * wt[None, :].astype(f32)
                 + bt[None, :].astype(f32)).astype(f32)     # [S-1, H]
    steps = [s for s in range(S - 1) if float(np.abs(tm[s]).max()) > 0.0]
    w3s = []
    hsc = []
    for s in steps:
        W3tm = W3 * tm[s][:, None]
        a3 = float(2.0 ** np.floor(np.log2(240.0 / np.abs(W3tm).max())))
        w3s.append(_fold_w_dr(W3tm * a3))
        hsc.append(float(dt) / a3)
    const["w3t8"] = _to_fp8(np.stack(w3s, axis=1))  # [128, NE, 2, 2, 256]
    flags = {
        "has_b1": bool(np.any(b1)),
        "has_aff1": bool(np.any(be1)) or bool(np.any(g1 != 1.0)),
        "has_b2": bool(np.any(b2)),
        "has_b3": bool(np.any(b3)),
        "has_affo": bool(np.any(be_out)) or bool(np.any(g_out != 1.0)),
    }
    if flags["has_b1"]:
        b1c = (b1 - b1.mean()) * A1
        const["b1cp"] = _fold_v(b1c.astype(f32))
    if flags["has_aff1"]:
        const["g1p"] = _fold_v(g1.astype(f32))
        const["be1p"] = _fold_v(be1.astype(f32))
    if flags["has_b2"]:
        const["b2p"] = _fold_v(b2.astype(f32))
    if flags["has_b3"]:
        import ml_dtypes
        const["b3row"] = np.ascontiguousarray(
            b3.astype(f32)[None, :]).astype(ml_dtypes.bfloat16)
    if flags["has_affo"]:
        const["goutp"] = _fold_v(g_out.astype(f32))
        const["beoutp"] = _fold_v(be_out.astype(f32))
    return const, flags, len(steps), tuple(hsc)


def shard_inputs(x, h, ncores):
    B = x.shape[0]
    BC = B // ncores
    per_core = []
    for i in range(ncores):
        sl = slice(i * BC, (i + 1) * BC)
        xc = _fold_bm(np.asarray(x[sl], dtype=np.float32))
        hc = _fold_bm(np.asarray(h[sl], dtype=np.float32))
        per_core.append({
            "x8": _to_fp8(xc),
            "hf": hc,
            "h8": _to_fp8(hc),
        })
    return per_core, BC


def _split_waits_bir(bir_bytes):
    """This container's walrus build rejects >1 embedded sync-wait per
    instruction (and any wait on a Drain). Spill surplus waits into bare
    EventSemaphore instructions just before the owner -- engine program order
    makes the ordering semantics identical, walrus encodes each happily."""
    import json
    m = json.loads(bir_bytes)
    cnt = 0
    for fn in m.get("functions", []):
        for blk in fn.get("blocks", []):
            out = []
            for inst in blk.get("instructions", []):
                body = inst
                si = body.get("sync_info")
                opcode = body.get("opcode")
                waits = (si or {}).get("on_wait") or []
                keep = 0 if opcode == "Drain" else 1
                if si and len(waits) > keep:
                    nspill = len(waits) - keep
                    for w in waits[:nspill]:
                        cnt += 1
                        out.append({
                            "name": f"WSPLIT-{cnt}",
                            "engine": body["engine"],
                            "opcode": "EventSemaphore",
                            "ins": [],
                            "outs": [],
                            "sync_info": {"on_wait": [w], "on_update": []},
                        })
                    si["on_wait"] = waits[nspill:]
                out.append(inst)
            blk["instructions"] = out
    return json.dumps(m).encode()


_BIR_FIX_DONE = False


def _install_bir_fix():
    global _BIR_FIX_DONE
    if _BIR_FIX_DONE:
        return
    import sys
    from concourse import bass_utils as bu

    orig = bu.compile_bir_kernel

    def wrapped(bir_json, tmpdir, neff_name="file.neff"):
        if isinstance(bir_json, str):
            bir_json = bir_json.encode()
        return orig(_split_waits_bir(bir_json), tmpdir, neff_name)

    bu.compile_bir_kernel = wrapped
    b2j = sys.modules.get("concourse.bass2jax")
    if b2j is not None and getattr(b2j, "compile_bir_kernel", None) is orig:
        b2j.compile_bir_kernel = wrapped

    if LDW_OPT:
        orig_run = bu.run_command

        def run2(cmd, **kw):
            if isinstance(cmd, list):
                cmd = ["--enable-ldw-opt=true"
                       if c == "--enable-ldw-opt=false" else c for c in cmd]
            return orig_run(cmd, **kw)

        bu.run_command = run2
    _BIR_FIX_DONE = True


_NC_CACHE = {}
TRACE = False          # set by test.py to capture an NTFF profile
LAST_RESULTS = None    # BassKernelResults of the most recent run


def kernel(x, h, W1, b1, g1, be1, W2, b2, W3, b3, wt, bt, g_out, be_out,
           adapt_steps):
    _install_bir_fix()
    from concourse import bass_utils

    S = int(adapt_steps)
    x = np.asarray(x)
    h = np.asarray(h)
    B, H = h.shape
    assert B % NCORES == 0

    f32 = np.float32
    tm = np.tanh(np.linspace(0.0, 1.0, S)[: S - 1, None].astype(f32)
                 * np.asarray(wt, f32)[None, :] + np.asarray(bt, f32)[None, :])
    if not np.any(tm):
        # every Euler step is multiplied by tm==0: output is just LN(h)
        hv = np.asarray(h, f32)
        mu = hv.mean(-1, keepdims=True)
        var = ((hv - mu) ** 2).mean(-1, keepdims=True)
        y = (hv - mu) / np.sqrt(var + LN_EPS)
        return (y * np.asarray(g_out, f32) + np.asarray(be_out, f32)
                ).astype(np.float32)

    const, flags, NE, hsc = prepare(
        np.asarray(W1, f32), np.asarray(b1, f32), np.asarray(g1, f32),
        np.asarray(be1, f32), np.asarray(W2, f32), np.asarray(b2, f32),
        np.asarray(W3, f32), np.asarray(b3, f32), np.asarray(wt, f32),
        np.asarray(bt, f32), np.asarray(g_out, f32), np.asarray(be_out, f32),
        S)

    shards, BC = shard_inputs(x, h, NCORES)

    key = (BC, NE, hsc, tuple(sorted(flags.items())))
    if key not in _NC_CACHE:
        shapes = {k: v.shape for k, v in const.items()}
        shapes.update({"x8": (P, 2, BC), "hf": (P, 2, BC), "h8": (P, 2, BC)})
        _NC_CACHE[key] = build_nc(BC, NE, hsc, flags, shapes)
    nc = _NC_CACHE[key]

    in_maps = [{**const, **sh} for sh in shards]
    res = bass_utils.run_bass_kernel_spmd(nc, in_maps,
                                          core_ids=list(range(NCORES)),
                                          trace=TRACE)
    global LAST_RESULTS
    LAST_RESULTS = res
    out = np.empty((B, H), dtype=np.float32)
    for i in range(NCORES):
        oc = res.results[i]["out"]                      # [128, 2, BC]
        out[i * BC:(i + 1) * BC] = oc.transpose(2, 1, 0).reshape(BC, H)
    return out


# revision 23
# speedup vs baseline: 1.0580x; 1.0225x over previous
"""Trainium2 Bass kernel v3: LiquidCell (Euler scan over 3-layer MLP+LN).

Changes vs v2 (4.10 ms):
- tm(t=0)=tanh(0)=0: the first Euler step contributes nothing -> run only
  the 8 steps with tm != 0 (host detects zero steps generically).
- Stale-R LayerNorm: normalize each step's L1 PSUM output directly with the
  previous step's 1/sigma (one fused DVE multiply per PSUM group replaces the
  drain + separate normalize passes), then refresh R afterward with the same
  self-correcting Newton form R' = R*(1.5 - 0.5*E[(z*R)^2]).  numpy sim of
  the full pipeline: rel_l2 1.2e-3 (v2 measured 2.2e-3).
- tanh(t*wt)*dt folded into per-step fp8 W3 copies -> h-update is one merged
  scalar_tensor_tensor with an immediate scale; no tmdtp tensor.
- Bigger instructions: one N=4096 gelu for L1, two N=1024 gelus for L2,
  N=1024 fused drains / h-update (PSUM tiles span 2 banks) to amortize the
  ~352-cycle ACT and ~130-cycle DVE fixed costs.
- Variance matmul: sampled squares stored fp8 (scale 1/16) so the reduction
  is a single DoubleRow fp8 matmul.
- Engine balance per tile-step: PE ~8.0us, DVE ~7.2us, ACT ~6.7us,
  GPSIMD ~5.6us.
"""

import numpy as np

P = 128
NCORES = 8
BT = 512          # batch tile (matmul free dim)
G = 4             # tiles interleaved per loop body
LN_EPS = 1e-5
A1 = 16.0         # host scale folded into W1 (cancels in LN)
A2 = 16.0         # host scale folded into W2 (divided out in gelu2 scale)
SS_CH = 2         # z1 chunks sampled for the LN variance estimate (of 8)
LDW_OPT = False   # --enable-ldw-opt=true crashes walrus codegen (CoreV3GenImpl)


def _emit(nc, tc, d, BC, NE, hsc, flags):
    import concourse.mybir as mybir
    from concourse.bass import ds, ts
    from contextlib import ExitStack

    f32 = mybir.dt.float32
    bf16 = mybir.dt.bfloat16
    fp8 = mybir.dt.float8e4
    AF = mybir.ActivationFunctionType
    OP = mybir.AluOpType
    ET = mybir.EngineType
    DR = mybir.MatmulPerfMode.DoubleRow

    has_b1 = flags["has_b1"]
    has_aff1 = flags["has_aff1"]
    has_b2 = flags["has_b2"]
    has_b3 = flags["has_b3"]
    has_affo = flags["has_affo"]

    n_m1 = 8                  # 4H/P
    n_gr = 4                  # L1 PSUM groups of 2 chunks
    n_m2 = 4                  # 2H/P
    n_c = 2                   # H/P

    with ExitStack() as ctx:
        singles = ctx.enter_context(tc.tile_pool(name="singles", bufs=1))
        pool_io = ctx.enter_context(tc.tile_pool(name="io", bufs=2 * G))
        pool_big = ctx.enter_context(tc.tile_pool(name="big", bufs=G))
        pool_w = ctx.enter_context(tc.tile_pool(name="wrk", bufs=4))
        pool_fin = ctx.enter_context(tc.tile_pool(name="fin", bufs=2))
        pool_row = ctx.enter_context(tc.tile_pool(name="rows", bufs=2))
        psA = ctx.enter_context(tc.tile_pool(name="psA", bufs=2, space="PSUM"))
        psB = ctx.enter_context(tc.tile_pool(name="psB", bufs=2, space="PSUM"))

        def load(name, dtype):
            t = singles.tile(list(d[name].shape), dtype, tag=name)
            nc.sync.dma_start(out=t, in_=d[name][:])
            return t

        w1t8 = load("w1t8", fp8)      # [128, 2, 2, 1024]
        w2t8 = load("w2t8", fp8)      # [128, 4, 2, 512]
        w3t8 = load("w3t8", fp8)      # [128, NE, 2, 2, 256]
        b1cp = load("b1cp", f32) if has_b1 else None
        g1p = load("g1p", f32) if has_aff1 else None
        be1p = load("be1p", f32) if has_aff1 else None
        b2p = load("b2p", f32) if has_b2 else None
        b3row = load("b3row", bf16) if has_b3 else None
        goutp = load("goutp", f32) if has_affo else None
        beoutp = load("beoutp", f32) if has_affo else None

        # seed-step variance row: vrow = A1^2 * sigma^2 over SS_CH*P features
        ones_ss = singles.tile([P, 1], bf16)
        nc.vector.memset(ones_ss, 1.0 / (SS_CH * P))
        # steady-state: q01 holds u^2/16; allones8 (fp8 DR lhsT) of 1/16 makes
        # bcv[p,j] = (1/256) sum_{256} u^2 = vhat (the 16s cancel)
        allones8 = singles.tile([P, 2, P], fp8)
        nc.vector.memset(allones8, 1.0 / 16.0)
        onerow = singles.tile([1, P], bf16)
        nc.vector.memset(onerow, 1.0)
        ones_hb = singles.tile([P, 1], bf16)
        nc.vector.memset(ones_hb, 1.0 / (n_c * P))
        eps_t = singles.tile([1, 1], f32)
        nc.vector.memset(eps_t, LN_EPS * A1 * A1)
        eps_o = singles.tile([1, 1], f32)
        nc.vector.memset(eps_o, LN_EPS)
        c15 = singles.tile([P, 1], f32)
        nc.vector.memset(c15, 1.5)
        dmy = singles.tile([1, 1], f32)
        if has_b3:
            onesrow_bf = singles.tile([1, BT], bf16)
            nc.vector.memset(onesrow_bf, 1.0)

        class T:
            pass

        def make_tile(off, j):
            t = T()
            t.off = off
            t.j = j
            t.x8 = pool_io.tile([P, 2, BT], fp8, tag="x8")
            t.hT = pool_io.tile([P, 2, BT], f32, tag="hT")
            t.h8 = pool_io.tile([P, 2, BT], fp8, tag="h8")
            t.u = pool_big.tile([P, n_m1, BT], bf16, tag="u")
            t.z1g = pool_big.tile([P, n_m1, BT], fp8, tag="z1g")
            t.z2 = pool_big.tile([P, n_m2, BT], fp8, tag="z2")
            t.q01 = pool_big.tile([P, 2, BT], fp8, tag="q01")
            t.rsB = pool_big.tile([P, BT], bf16, tag="rsB")
            nc.sync.dma_start(out=t.x8, in_=d["x8"][:, :, ds(off, BT)])
            nc.sync.dma_start(out=t.hT, in_=d["hf"][:, :, ds(off, BT)])
            nc.sync.dma_start(out=t.h8, in_=d["h8"][:, :, ds(off, BT)])
            return t

        def l1_mms(t, p):
            # one PSUM group = out-chunks 2p, 2p+1; h-pass + x-pass per chunk
            zps = psA.tile([P, 2, BT], f32, tag="l1")
            for c in range(2):
                m = 2 * p + c
                nc.tensor.matmul(zps[:, c, :], lhsT=w1t8[:, 0, :, ts(m, P)],
                                 rhs=t.h8, start=True, stop=False, perf_mode=DR)
                nc.tensor.matmul(zps[:, c, :], lhsT=w1t8[:, 1, :, ts(m, P)],
                                 rhs=t.x8, start=False, stop=True, perf_mode=DR)
            return zps

        def fused_drain(t, p, zps):
            # u = (z + b1) * R  with R from the previous step (stale)
            sl = slice(2 * p, 2 * p + 2)
            rb = t.rsB[:, None, :].to_broadcast((P, 2, BT))
            if has_b1:
                for c in range(2):
                    m = 2 * p + c
                    nc.vector.scalar_tensor_tensor(
                        out=t.u[:, m, :], in0=zps[:, c, :],
                        scalar=b1cp[:, m:m + 1], in1=t.rsB,
                        op0=OP.add, op1=OP.mult)
            else:
                nc.vector.tensor_mul(out=t.u[:, sl, :], in0=zps, in1=rb)

        def sq(t):
            # q01 = u^2/16 (fp8); sampled squares for the variance estimate
            nc.vector.scalar_tensor_tensor(
                out=t.q01, in0=t.u[:, 0:2, :], scalar=1.0 / 16.0,
                in1=t.u[:, 0:2, :], op0=OP.mult, op1=OP.mult)

        def pWB(t):
            # emitted one slot after sq: bcv never blocks the PE FIFO and
            # w is the first ACT op of the slot
            bcv = psB.tile([P, 2, BT], f32, tag="ps2")
            nc.tensor.matmul(bcv[:, 0, :], lhsT=allones8, rhs=t.q01,
                             start=True, stop=True, perf_mode=DR)
            # w = 1.5 - 0.5*vhat ; R <- R*w   (one-Newton rsqrt at seed 1)
            t.wP = pool_w.tile([P, BT], bf16, tag="wP")
            nc.scalar.activation(out=t.wP, in_=bcv[:, 0, :],
                                 func=AF.Identity, bias=c15[:, 0:1],
                                 scale=-0.5)

        def r_mul(t):
            nc.gpsimd.tensor_mul(out=t.rsB, in0=t.rsB, in1=t.wP)

        def gelu1(t):
            if has_aff1:
                for m in range(n_m1):
                    nc.scalar.activation(out=t.z1g[:, m, :], in_=t.u[:, m, :],
                                         func=AF.Gelu, bias=be1p[:, m:m + 1],
                                         scale=g1p[:, m:m + 1])
            else:
                nc.scalar.activation(out=t.z1g, in_=t.u, func=AF.Gelu,
                                     bias=0.0, scale=1.0)

        def pA_seed(t):
            # first real step: raw drains, exact ln/exp 1/sigma, normalize.
            # dummy Ln up front: the ACT table load runs during the L1
            # matmuls instead of sitting on the serial stats chain
            nc.scalar.activation(out=dmy, in_=eps_o[0:1, 0:1], func=AF.Ln,
                                 bias=0.0, scale=1.0)
            for p in range(n_gr):
                zps = l1_mms(t, p)
                if p % 2 == 0:
                    nc.scalar.copy(out=t.u[:, 2 * p:2 * p + 2, :], in_=zps)
                else:
                    nc.vector.tensor_copy(out=t.u[:, 2 * p:2 * p + 2, :],
                                          in_=zps)
            q01b = pool_fin.tile([P, 2, BT], bf16, tag="q0b")
            nc.vector.tensor_mul(out=q01b, in0=t.u[:, 0:2, :],
                                 in1=t.u[:, 0:2, :])
            vps = psB.tile([P, 2, BT], f32, tag="ps2")
            for m in range(SS_CH):
                nc.tensor.matmul(vps[0:1, 0, :], lhsT=ones_ss,
                                 rhs=q01b[:, m, :], start=(m == 0),
                                 stop=(m == SS_CH - 1))
            lnv = pool_row.tile([1, BT], f32, tag="lnv")
            nc.scalar.activation(out=lnv, in_=vps[0:1, 0, :], func=AF.Ln,
                                 bias=eps_t[0:1, 0:1], scale=1.0)
            rsr = pool_row.tile([1, BT], bf16, tag="rsr")
            nc.scalar.activation(out=rsr, in_=lnv, func=AF.Exp,
                                 bias=0.0, scale=-0.5)
            bc = psB.tile([P, 2, BT], f32, tag="ps2")
            nc.tensor.matmul(bc[:, 0, :], lhsT=onerow, rhs=rsr,
                             start=True, stop=True)
            nc.vector.tensor_copy(out=t.rsB, in_=bc[:, 0, :])
            rb = t.rsB[:, None, :].to_broadcast((P, 4, BT))
            for hf in range(2):
                hs = slice(4 * hf, 4 * hf + 4)
                if has_b1:
                    for m in range(4 * hf, 4 * hf + 4):
                        nc.vector.scalar_tensor_tensor(
                            out=t.u[:, m, :], in0=t.u[:, m, :],
                            scalar=b1cp[:, m:m + 1], in1=t.rsB,
                            op0=OP.add, op1=OP.mult)
                else:
                    nc.vector.tensor_mul(out=t.u[:, hs, :], in0=t.u[:, hs, :],
                                         in1=rb)
                # gelu each half right after its normalize: the Gelu table
                # reload hides under the second normalize multiply
                gelu1_half(t, hf)

        def gelu1_half(t, hf):
            hs = slice(4 * hf, 4 * hf + 4)
            if has_aff1:
                for m in range(4 * hf, 4 * hf + 4):
                    nc.scalar.activation(out=t.z1g[:, m, :], in_=t.u[:, m, :],
                                         func=AF.Gelu, bias=be1p[:, m:m + 1],
                                         scale=g1p[:, m:m + 1])
            else:
                nc.scalar.activation(out=t.z1g[:, hs, :], in_=t.u[:, hs, :],
                                     func=AF.Gelu, bias=0.0, scale=1.0)

        def pA(t, seed, last):
            if seed:
                pA_seed(t)
                return
            # gelu1 in halves so z1g completes ~1.7us after the last drain
            # instead of ~3.7us; PE/DVE emission order is untouched
            for p in range(n_gr):
                zps = l1_mms(t, p)
                fused_drain(t, p, zps)
                if p == 0 and not last:
                    sq(t)
                if p == 1:
                    gelu1_half(t, 0)
            gelu1_half(t, 1)

        def pBC(t, i, seed, last):
            # L2 in two halves, straight from PSUM through gelu2
            for h in range(2):
                zps = psB.tile([P, 2, BT], f32, tag="ps2")
                for c in range(2):
                    m = 2 * h + c
                    for p8 in range(4):
                        nc.tensor.matmul(zps[:, c, :],
                                         lhsT=w2t8[:, p8, :, ts(m, P)],
                                         rhs=t.z1g[:, 2 * p8:2 * p8 + 2, :],
                                         start=(p8 == 0), stop=(p8 == 3),
                                         perf_mode=DR)
                if has_b2:
                    for c in range(2):
                        m = 2 * h + c
                        nc.scalar.activation(out=t.z2[:, m, :],
                                             in_=zps[:, c, :], func=AF.Gelu,
                                             bias=b2p[:, m:m + 1],
                                             scale=1.0 / A2)
                else:
                    nc.scalar.activation(out=t.z2[:, 2 * h:2 * h + 2, :],
                                         in_=zps, func=AF.Gelu,
                                         bias=0.0, scale=1.0 / A2)
            # L3 with per-step W3*tm*dt folded weights
            dps = psB.tile([P, 2, BT], f32, tag="ps2")
            for c in range(n_c):
                for p4 in range(2):
                    nc.tensor.matmul(dps[:, c, :],
                                     lhsT=w3t8[:, i, p4, :, ts(c, P)],
                                     rhs=t.z2[:, 2 * p4:2 * p4 + 2, :],
                                     start=(p4 == 0),
                                     stop=(p4 == 1) and not has_b3,
                                     perf_mode=DR)
                if has_b3:
                    nc.tensor.matmul(dps[:, c, :], lhsT=b3row[0:1, ts(c, P)],
                                     rhs=onesrow_bf, start=False, stop=True)
            nc.vector.scalar_tensor_tensor(
                out=t.hT, in0=dps, scalar=hsc[i], in1=t.hT,
                op0=OP.mult, op1=OP.add)
            if not last:
                nc.gpsimd.tensor_copy(out=t.h8, in_=t.hT)
                if not seed:
                    r_mul(t)

        def pfinal(t):
            hb = pool_fin.tile([P, 2, BT], bf16, tag="hb")
            nc.gpsimd.tensor_copy(out=hb, in_=t.hT)
            q2 = pool_fin.tile([P, 2, BT], bf16, tag="q2")
            nc.vector.tensor_mul(out=q2, in0=hb, in1=hb)
            rows = psB.tile([P, 2, BT], f32, tag="ps2")
            for c in range(n_c):
                nc.tensor.matmul(rows[0:1, 0, :], lhsT=ones_hb,
                                 rhs=hb[:, c, :], start=(c == 0),
                                 stop=(c == n_c - 1))
            for c in range(n_c):
                nc.tensor.matmul(rows[0:1, 1, :], lhsT=ones_hb,
                                 rhs=q2[:, c, :], start=(c == 0),
                                 stop=(c == n_c - 1))
            mu2 = pool_row.tile([1, BT], f32, tag="mu2")
            nc.vector.tensor_copy(out=mu2, in_=rows[0:1, 0, :])
            var2 = pool_row.tile([1, BT], f32, tag="var2")
            msq = pool_row.tile([1, BT], f32, tag="msq")
            nc.vector.tensor_mul(out=msq, in0=mu2, in1=mu2)
            nc.vector.tensor_sub(out=var2, in0=rows[0:1, 1, :], in1=msq)
            lnv2 = pool_row.tile([1, BT], f32, tag="lnv2")
            nc.scalar.activation(out=lnv2, in_=var2, func=AF.Ln,
                                 bias=eps_o[0:1, 0:1], scale=1.0)
            rs2 = pool_row.tile([1, BT], bf16, tag="rs2")
            nc.scalar.activation(out=rs2, in_=lnv2, func=AF.Exp,
                                 bias=0.0, scale=-0.5)
            mu2b = pool_row.tile([1, BT], bf16, tag="mu2b")
            nc.vector.tensor_copy(out=mu2b, in_=mu2)
            bmrs = psB.tile([P, 2, BT], f32, tag="ps2")
            nc.tensor.matmul(bmrs[:, 0, :], lhsT=onerow, rhs=mu2b,
                             start=True, stop=True)
            nc.tensor.matmul(bmrs[:, 1, :], lhsT=onerow, rhs=rs2,
                             start=True, stop=True)
            dd = pool_fin.tile([P, 2, BT], f32, tag="dd")
            nc.vector.tensor_sub(
                out=dd, in0=t.hT,
                in1=bmrs[:, 0:1, :].to_broadcast((P, 2, BT)))
            nc.vector.tensor_mul(
                out=dd, in0=dd,
                in1=bmrs[:, 1:2, :].to_broadcast((P, 2, BT)))
            if has_affo:
                oc = pool_fin.tile([P, 2, BT], f32, tag="oc")
                for c in range(n_c):
                    nc.scalar.activation(out=oc[:, c, :], in_=dd[:, c, :],
                                         func=AF.Identity,
                                         bias=beoutp[:, c:c + 1],
                                         scale=goutp[:, c:c + 1])
                dd = oc
            nc.scalar.dma_start(out=d["out"][:, :, ds(t.off, BT)], in_=dd)

        assert BC % (G * BT) == 0
        NSEG = BC // (G * BT)
        # fully unrolled flat pipeline over NSEG*NE*G tile-slots: pBC lags pA
        # by 2 slots, pWB by 1, so each step's L2/L3 tail interleaves with the
        # next step's L1 and the h-update STTs never sit ahead of the next
        # drains in DVE's FIFO.  No hardware loop: tc.For_i drains every
        # engine queue at each wrap (a full barrier), so straight-line code
        # removes those pipeline flushes entirely.
        slots = [(b, i, j) for b in range(NSEG)
                 for i in range(NE) for j in range(G)]
        tiles = {}
        for k in range(len(slots) + 2):
            if k < len(slots):
                b, i, j = slots[k]
                if i == 0:
                    tiles[(b, j)] = make_tile((b * G + j) * BT, j)
                pA(tiles[(b, j)], i == 0, i == NE - 1)
            if 1 <= k <= len(slots):
                b, i, j = slots[k - 1]
                if 0 < i < NE - 1:
                    pWB(tiles[(b, j)])
            if k >= 2:
                b, i, j = slots[k - 2]
                pBC(tiles[(b, j)], i, i == 0, i == NE - 1)
                if i == NE - 1:
                    pfinal(tiles[(b, j)])


def build_nc(BC, NE, hsc, flags, shapes):
    import concourse.bass as bass
    import concourse.mybir as mybir
    import concourse.tile as tile

    f32 = mybir.dt.float32
    bf16 = mybir.dt.bfloat16
    fp8 = mybir.dt.float8e4
    nc = bass.Bass("TRN2", target_bir_lowering=False, debug=False)
    d = {}
    dts = {"x8": fp8, "hf": f32, "h8": fp8, "w1t8": fp8, "w2t8": fp8,
           "w3t8": fp8, "goutp": f32, "beoutp": f32,
           "b1cp": f32, "g1p": f32, "be1p": f32, "b2p": f32, "b3row": bf16}
    for name, shape in shapes.items():
        d[name] = nc.dram_tensor(name, list(shape), dts[name],
                                 kind="ExternalInput")[:]
    d["out"] = nc.dram_tensor("out", [P, 2, BC], f32, kind="ExternalOutput")[:]
    with tile.TileContext(nc) as tc:
        _emit(nc, tc, d, BC, NE, hsc, flags)
    return nc


def _fold_w_dr(W):
    """W [M, K] -> DoubleRow lhsT tiles [128, K/256, 2, M]:
    [kp, pr, i, j] = W[j, (2*pr+i)*128 + kp]."""
    M, K = W.shape
    return np.ascontiguousarray(
        W.T.reshape(K // 256, 2, P, M).transpose(2, 0, 1, 3))


def _fold_v(v):
    """v [F] -> [128, F/128] with [p, c] = v[c*128+p]."""
    return np.ascontiguousarray(v.reshape(-1, P).T)


def _fold_bm(a):
    """a [N, F] batch-major -> feature-major [128, F/128, N]."""
    n, f = a.shape
    return np.ascontiguousarray(a.T.reshape(f // P, P, n).transpose(1, 0, 2))


def _to_fp8(a):
    import ml_dtypes
    return np.clip(a, -240.0, 240.0).astype(ml_dtypes.float8_e4m3)


def prepare(W1, b1, g1, be1, W2, b2, W3, b3, wt, bt, g_out, be_out, S):
    f32 = np.float32
    const = {}
    W1c = (W1 - W1.mean(axis=0, keepdims=True)) * A1
    const["w1t8"] = _to_fp8(_fold_w_dr(W1c))
    const["w2t8"] = _to_fp8(_fold_w_dr(W2 * A2))
    ts_ = np.linspace(0.0, 1.0, S).astype(f32)
    dt = ts_[1] - ts_[0]
    tm = np.tanh(ts_[: S - 1, None] * wt[None, :].astype(f32)
                 + bt[None, :].astype(f32)).astype(f32)     # [S-1, H]
    steps = [s for s in range(S - 1) if float(np.abs(tm[s]).max()) > 0.0]
    w3s = []
    hsc = []
    for s in steps:
        W3tm = W3 * tm[s][:, None]
        a3 = float(2.0 ** np.floor(np.log2(240.0 / np.abs(W3tm).max())))
        w3s.append(_fold_w_dr(W3tm * a3))
        hsc.append(float(dt) / a3)
    const["w3t8"] = _to_fp8(np.stack(w3s, axis=1))  # [128, NE, 2, 2, 256]
    flags = {
        "has_b1": bool(np.any(b1)),
        "has_aff1": bool(np.any(be1)) or bool(np.any(g1 != 1.0)),
        "has_b2": bool(np.any(b2)),
        "has_b3": bool(np.any(b3)),
        "has_affo": bool(np.any(be_out)) or bool(np.any(g_out != 1.0)),
    }
    if flags["has_b1"]:
        b1c = (b1 - b1.mean()) * A1
        const["b1cp"] = _fold_v(b1c.astype(f32))
    if flags["has_aff1"]:
        const["g1p"] = _fold_v(g1.astype(f32))
        const["be1p"] = _fold_v(be1.astype(f32))
    if flags["has_b2"]:
        const["b2p"] = _fold_v(b2.astype(f32))
    if flags["has_b3"]:
        import ml_dtypes
        const["b3row"] = np.ascontiguousarray(
            b3.astype(f32)[None, :]).astype(ml_dtypes.bfloat16)
    if flags["has_affo"]:
        const["goutp"] = _fold_v(g_out.astype(f32))
        const["beoutp"] = _fold_v(be_out.astype(f32))
    return const, flags, len(steps), tuple(hsc)


def shard_inputs(x, h, ncores):
    B = x.shape[0]
    BC = B // ncores
    per_core = []
    for i in range(ncores):
        sl = slice(i * BC, (i + 1) * BC)
        xc = _fold_bm(np.asarray(x[sl], dtype=np.float32))
        hc = _fold_bm(np.asarray(h[sl], dtype=np.float32))
        per_core.append({
            "x8": _to_fp8(xc),
            "hf": hc,
            "h8": _to_fp8(hc),
        })
    return per_core, BC


def _split_waits_bir(bir_bytes):
    """This container's walrus build rejects >1 embedded sync-wait per
    instruction (and any wait on a Drain). Spill surplus waits into bare
    EventSemaphore instructions just before the owner -- engine program order
    makes the ordering semantics identical, walrus encodes each happily."""
    import json
    m = json.loads(bir_bytes)
    cnt = 0
    for fn in m.get("functions", []):
        for blk in fn.get("blocks", []):
            out = []
            for inst in blk.get("instructions", []):
                body = inst
                si = body.get("sync_info")
                opcode = body.get("opcode")
                waits = (si or {}).get("on_wait") or []
                keep = 0 if opcode == "Drain" else 1
                if si and len(waits) > keep:
                    nspill = len(waits) - keep
                    for w in waits[:nspill]:
                        cnt += 1
                        out.append({
                            "name": f"WSPLIT-{cnt}",
                            "engine": body["engine"],
                            "opcode": "EventSemaphore",
                            "ins": [],
                            "outs": [],
                            "sync_info": {"on_wait": [w], "on_update": []},
                        })
                    si["on_wait"] = waits[nspill:]
                out.append(inst)
            blk["instructions"] = out
    return json.dumps(m).encode()


_BIR_FIX_DONE = False


def _install_bir_fix():
    global _BIR_FIX_DONE
    if _BIR_FIX_DONE:
        return
    import sys
    from concourse import bass_utils as bu

    orig = bu.compile_bir_kernel

    def wrapped(bir_json, tmpdir, neff_name="file.neff"):
        if isinstance(bir_json, str):
            bir_json = bir_json.encode()
        return orig(_split_waits_bir(bir_json), tmpdir, neff_name)

    bu.compile_bir_kernel = wrapped
    b2j = sys.modules.get("concourse.bass2jax")
    if b2j is not None and getattr(b2j, "compile_bir_kernel", None) is orig:
        b2j.compile_bir_kernel = wrapped

    if LDW_OPT:
        orig_run = bu.run_command

        def run2(cmd, **kw):
            if isinstance(cmd, list):
                cmd = ["--enable-ldw-opt=true"
                       if c == "--enable-ldw-opt=false" else c for c in cmd]
            return orig_run(cmd, **kw)

        bu.run_command = run2
    _BIR_FIX_DONE = True


_NC_CACHE = {}
TRACE = False          # set by test.py to capture an NTFF profile
LAST_RESULTS = None    # BassKernelResults of the most recent run


def kernel(x, h, W1, b1, g1, be1, W2, b2, W3, b3, wt, bt, g_out, be_out,
           adapt_steps):
    _install_bir_fix()
    from concourse import bass_utils

    S = int(adapt_steps)
    x = np.asarray(x)
    h = np.asarray(h)
    B, H = h.shape
    assert B % NCORES == 0

    f32 = np.float32
    tm = np.tanh(np.linspace(0.0, 1.0, S)[: S - 1, None].astype(f32)
                 * np.asarray(wt, f32)[None, :] + np.asarray(bt, f32)[None, :])
    if not np.any(tm):
        # every Euler step is multiplied by tm==0: output is just LN(h)
        hv = np.asarray(h, f32)
        mu = hv.mean(-1, keepdims=True)
        var = ((hv - mu) ** 2).mean(-1, keepdims=True)
        y = (hv - mu) / np.sqrt(var + LN_EPS)
        return (y * np.asarray(g_out, f32) + np.asarray(be_out, f32)
                ).astype(np.float32)

    const, flags, NE, hsc = prepare(
        np.asarray(W1, f32), np.asarray(b1, f32), np.asarray(g1, f32),
        np.asarray(be1, f32), np.asarray(W2, f32), np.asarray(b2, f32),
        np.asarray(W3, f32), np.asarray(b3, f32), np.asarray(wt, f32),
        np.asarray(bt, f32), np.asarray(g_out, f32), np.asarray(be_out, f32),
        S)

    shards, BC = shard_inputs(x, h, NCORES)

    key = (BC, NE, hsc, tuple(sorted(flags.items())))
    if key not in _NC_CACHE:
        shapes = {k: v.shape for k, v in const.items()}
        shapes.update({"x8": (P, 2, BC), "hf": (P, 2, BC), "h8": (P, 2, BC)})
        _NC_CACHE[key] = build_nc(BC, NE, hsc, flags, shapes)
    nc = _NC_CACHE[key]

    in_maps = [{**const, **sh} for sh in shards]
    res = bass_utils.run_bass_kernel_spmd(nc, in_maps,
                                          core_ids=list(range(NCORES)),
                                          trace=TRACE)
    global LAST_RESULTS
    LAST_RESULTS = res
    out = np.empty((B, H), dtype=np.float32)
    for i in range(NCORES):
        oc = res.results[i]["out"]                      # [128, 2, BC]
        out[i * BC:(i + 1) * BC] = oc.transpose(2, 1, 0).reshape(BC, H)
    return out
